# revision 17
# baseline (speedup 1.0000x reference)
"""Trainium2 Bass kernel for nn_EncoderLayer_73315091743398.

The reference module's attention einsums ('hwink,hwijm->hwinm') sum their k/j
indices independently, so the whole attention block collapses to, per
(h,w)-chunk c and head i, over the flat q matrix qf = x@Wq.T + pe viewed as
(8192, 512) in raw (s,h,w) row order:

    u[s]  = sum_d qf[c*512+s, 64i+d]          (segment row sums)
    a     = softmax_s(u)
    v[d]  = sum_s a[s] * qf[c*512+s, 64i+d]
    row   = tile8(v) @ Wfc.T = v @ M,  M[d,:] = sum_b Wfc[:, 64b+d].T

and attn_out viewed (S,H,W,D) has row A[s'] = row_{c=s'//32, i=(s'%32)//4},
independent of (h,w).  Core k owns raw rows [k*1024,(k+1)*1024): these are
exactly attention chunks {2k, 2k+1} AND the residual/FFN rows for
s' in [64k, 64k+64), so the 8 cores run fully independent SPMD programs.

v3 structure (vs the 120.7us baseline):
  - uT[i,s] is computed DIRECTLY as x @ Wu (Wu = segment-summed Wq.T
    columns) into an [8,512] PSUM per chunk + an exact f32 pe-segment-sum
    added on the DVE; softmax for both chunks runs early and entirely off
    the FFN critical path.
  - The value pass never materializes q: by associativity,
      v_i = (sum_s a_i[s] x[s,:]) @ Wq.T + (sum_s a_i[s] pe[s,:])
    so per chunk: xa = aT.T @ x (4 matmuls), v = xaT @ WqT (4) plus the
    pe term via group-sum selectors (8 tiny).  This deletes the 40 q
    matmuls and the 8 scalar-engine PSUM->SBUF q copies of the q path.
  - 1/sum(exp) is folded into the las row scale (tensor_scalar_mul),
    removing the ex normalization from the a-tensor critical path.
  - Exp and Sqrt ACT table sets would thrash (different sets, ~2.7us per
    switch): both softmax Exps are issued back-to-back, then a junk Sqrt
    that READS the second exp's output (ordering-proof) preloads the
    sqrt set once for all 16 LN chains.
  - o1T for the FFN is built with 4 PE transposes per tile into one PSUM
    bank + a single copy, replacing DMA transposes (1.2-1.5us descriptor
    stalls on the HWDGE queues).
  - Input DMA descriptor generation (~0.6-0.8us each on an HWDGE queue!)
    is split across BOTH queues (sync + scalar), small consts are packed
    into two tensors (bitcast views), and w1/w2 arrive in per-ft chunks
    so the FFN streams against DMA arrival.
  - Trivial LN affine params / biases (true for this problem's inputs)
    drop the GpSimd scale/bias ops; output is bf16, upcast on host.
"""

import math
import os
import sys
from contextlib import ExitStack

import numpy as np
import ml_dtypes  # noqa: F401  (registers bfloat16)

for _p in ("/opt/trn_rl_repo", "/root/.axon_site/_ro/trn_rl_repo"):
    if os.path.isdir(_p) and _p not in sys.path:
        sys.path.append(_p)

import concourse.bass as bass
import concourse.bacc as bacc
import concourse.mybir as mybir
import concourse.tile as tile
from concourse.bass_utils import run_bass_kernel_spmd

F32 = mybir.dt.float32
F16 = mybir.dt.float16
BF16 = mybir.dt.bfloat16
AF = mybir.ActivationFunctionType
ALU = mybir.AluOpType
AX = mybir.AxisListType

S, H, W, D = 512, 4, 4, 512
NH, DEP, DFF = 8, 64, 2048
NCORES = 8
R = 1024          # rows per core of the flat (8192, 512) view
EPS = 1e-5

# CF (f32): eye128 [128], b1p [16], wu-bitcast [16] -> 160
O_EYE, O_B1P, O_WU = 0, 128, 144
NCF = 160
# CB (bf16): eye128 + Mst + selTTb [8]
O_EYB, O_MST, O_SELB = 0, 128, 640
NCB = 648
# CS8 (bf16, 8 partitions): E8 selector rows + rows of g1/g2/be2/(b2+be1)
O_E8, O_G1R, O_G2R, O_BE2R, O_B1TR = 0, 512, 1024, 1536, 2048
NCS = 2560
# CR8 (fp16, 8 partitions): selT [8,128], peT [8,8*512], eye8 [8,8],
# peuT-bitcast [8, 2*512 f32]
O_SEL, O_PET, O_EY8, O_PEU = 0, 128, 128 + 8 * D, 128 + 8 * D + 8
NCR8 = O_PEU + 4 * D          # 2*512 f32 as 4*512 f16 halves

_cached = {}


def build_nc(trivial):
    """Single-core SPMD Bass/Tile program (same program on all 8 cores).

    trivial=True: g1==1, be1==0, b2==0, g2==1, be2==0 (the actual inputs);
    drops the residual scale and the output scale/bias ops.
    """
    nc = bacc.Bacc("TRN2", debug=False, target_bir_lowering=False)

    xT = nc.dram_tensor("xT", [D, R], F16, kind="ExternalInput")
    xRb = nc.dram_tensor("xRb", [128, 8 * D], BF16, kind="ExternalInput")
    WqT = nc.dram_tensor("WqT", [D, D], F16, kind="ExternalInput")
    W1T = nc.dram_tensor("W1T", [D, DFF], BF16, kind="ExternalInput")
    W2T = nc.dram_tensor("W2T", [DFF, D], BF16, kind="ExternalInput")
    CF = nc.dram_tensor("CF", [128, NCF], F32, kind="ExternalInput")
    CB = nc.dram_tensor("CB", [128, NCB], BF16, kind="ExternalInput")
    CS8 = nc.dram_tensor("CS8", [8, NCS], BF16, kind="ExternalInput")
    CR8 = nc.dram_tensor("CR8", [8, NCR8], F16, kind="ExternalInput")
    out = nc.dram_tensor("out", [R, D], BF16, kind="ExternalOutput")

    with ExitStack() as ctx:
        tc = ctx.enter_context(tile.TileContext(nc))
        cst = ctx.enter_context(tc.tile_pool(name="cst", bufs=1))
        xp = ctx.enter_context(tc.tile_pool(name="xp", bufs=1))
        qp = ctx.enter_context(tc.tile_pool(name="qp", bufs=1))
        hp = ctx.enter_context(tc.tile_pool(name="hp", bufs=1))
        wk = ctx.enter_context(tc.tile_pool(name="wk", bufs=2))
        ps = ctx.enter_context(tc.tile_pool(name="ps", bufs=1, space="PSUM"))

        # ---- loads, split across BOTH HWDGE queues (descriptor generation
        #      is ~0.6-0.8us each and would serialize on one queue).
        # sync queue: cr8(+pe sums), xq0..3, xr halves, w2 chunks
        # scalar queue: cf(+wu), wq, cb, cs8, w1 chunks
        cr8 = cst.tile([8, NCR8], F16, tag="cr8", name="cr8")
        nc.sync.dma_start(cr8[:], CR8[:])
        cf = cst.tile([128, NCF], F32, tag="cf", name="cf")
        nc.scalar.dma_start(cf[:], CF[:])
        xq = [xp.tile([128, R], F16, tag=f"dT{i}", name=f"xq{i}")
              for i in range(4)]
        for i in range(4):
            nc.sync.dma_start(xq[i][:], xT[i * 128:(i + 1) * 128, :])
        wq_all = cst.tile([128, 4 * D], F16, tag="wq", name="wq_all")
        nc.scalar.dma_start(
            wq_all[:].rearrange("p (t j) -> p t j", t=4),
            WqT.rearrange("(t p) j -> p t j", p=128))
        cb = cst.tile([128, NCB], BF16, tag="cb", name="cb")
        nc.scalar.dma_start(cb[:], CB[:])
        cs8 = cst.tile([8, NCS], BF16, tag="cs8", name="cs8")
        nc.scalar.dma_start(cs8[:], CS8[:])
        xr_all = xp.tile([128, 8 * D], BF16, tag="xr", name="xr_all")
        nc.sync.dma_start(xr_all[:, 0:4 * D], xRb[:, 0:4 * D])
        nc.sync.dma_start(xr_all[:, 4 * D:], xRb[:, 4 * D:])
        # w1 in 2-ft chunks; descriptors are ISSUED LATER (closures invoked
        # at schedule points) so they don't block the softmax ACTs on the
        # scalar HWDGE queue
        w1_all = cst.tile([128, 4 * DFF], BF16, tag="w1", name="w1_all")
        w1v = w1_all[:].rearrange("p (t f j) -> p t f j", t=4, f=8)
        w1s = W1T.rearrange("(t p) (f j) -> p t f j", p=128, f=8)
        w1_load = [lambda ft=ft: nc.scalar.dma_start(
            w1v[:, :, ft, :], w1s[:, :, ft, :]) for ft in range(8)]
        # w2 in 4-ft chunks on the sync queue (needed last)
        w2_all = cst.tile([128, 16 * D], BF16, tag="w2", name="w2_all")
        w2v = w2_all[:].rearrange("p (f d) -> p f d", f=16)
        w2s = W2T.rearrange("(f p) d -> p f d", p=128)
        for ft in range(0, 16, 4):
            nc.sync.dma_start(w2v[:, ft:ft + 4, :], w2s[:, ft:ft + 4, :])

        eye_sb = cf[:, O_EYE:O_EYE + 128]
        wu_sb = cf[:, O_WU:O_WU + 16].bitcast(F16)        # [128, 32] f16
        cu8 = cr8[:, O_PEU:O_PEU + 4 * D].bitcast(F32)    # [8, 1024] f32
        eye8h = cr8[0:8, O_EY8:O_EY8 + 8]                 # [8, 8] f16
        Mstcb = cb[:, O_MST:O_MST + D]
        EYBcb = cb[:, O_EYB:O_EYB + 128]
        selTTb = cb[:, O_SELB:O_SELB + 8]                 # [128, 8] bf16
        epsT = cst.tile([128, 1], F32, tag="eps", name="epsT")
        nc.vector.memset(epsT[:], EPS)

        if not trivial:
            G1cb = cst.tile([128, D], BF16, tag="g1t", name="g1t")
            G2cb = cst.tile([128, D], BF16, tag="g2t", name="g2t")
            BE2cb = cst.tile([128, D], BF16, tag="be2t", name="be2t")
            B1Tcb = cst.tile([128, D], BF16, tag="b1tt", name="b1tt")
            for bt, off in ((G1cb, O_G1R), (G2cb, O_G2R),
                            (BE2cb, O_BE2R), (B1Tcb, O_B1TR)):
                nc.gpsimd.partition_broadcast(bt[:], cs8[0:1, off:off + D])

        # ---- ACT table preload (exp set) + PE warm-up during DMA wait.
        # N=512 warm matmuls: only a sustained wide stream flips HAM to
        # 8/8 (N=64 junk measurably never does); later warm_fill() calls
        # bridge dependency stalls in the attention phase so the clock
        # never re-throttles before the FFN stream takes over.
        junk = cst.tile([128, 1], F32, tag="junk", name="junk")
        nc.scalar.activation(junk[:], epsT[:], AF.Exp)
        warm_sb = cst.tile([128, D], BF16, tag="wrm", name="warm_sb")
        nc.vector.memset(warm_sb[:], 0.0)

        def warm_fill(n, cols=256):
            wt = ps.tile([128, D], F32, tag="vc", bufs=4)
            for _ in range(n):
                nc.tensor.matmul(wt[:, 0:cols], warm_sb[:, 0:128],
                                 warm_sb[:, 0:cols], start=True, stop=True)

        warm_fill(10, cols=512)

        nrm1 = [qp.tile([128, D], BF16, tag=f"n1{m}", name=f"nrm1_{m}")
                for m in range(8)]
        if not trivial:
            o1_sb = [qp.tile([128, D], BF16, tag=f"o1{m}", name=f"o1sb{m}")
                     for m in range(8)]
        else:
            o1_sb = nrm1
        # o1T_all[p, m*512 + t*128 + y] = nrm1[m][y, t*128 + p]
        o1T_all = qp.tile([128, 8 * D], BF16, tag="oT", name="o1T_all")

        uT_sb = [qp.tile([8, D], F32, tag=f"uT{c}", name=f"uTsb{c}")
                 for c in range(2)]

        def uT_mm(c):
            """uT_ps[c][i, st*128+y] = sum_K x[row, K] * Wu[K, i] for the
            four tiles st of chunk c (cols 256*j from xq[2c+j])."""
            ups = ps.tile([8, D], F32, tag="mmA", bufs=2)
            # j=1 range: start=False on untouched PSUM (has_written unset
            # -> overwrite), so j=0's accumulating values aren't clobbered
            for j in range(2):
                for t in range(4):
                    nc.tensor.matmul(
                        ups[:, j * 256:(j + 1) * 256],
                        wu_sb[:, t * 8:(t + 1) * 8],
                        xq[2 * c + j][:, t * 256:(t + 1) * 256],
                        start=(j == 0 and t == 0), stop=(t == 3),
                        skip_group_check=True)
            # exact pe segment sums added on DVE (f16 would cost ~0.02 abs)
            nc.vector.tensor_add(uT_sb[c][:], ups[:],
                                 cu8[:, c * D:(c + 1) * D])

        def attn_softmax(c):
            mx = wk.tile([8, 1], F32, tag="mx")
            nc.vector.tensor_reduce(mx[:], uT_sb[c][:], axis=AX.X, op=ALU.max)
            nmx = wk.tile([8, 1], F32, tag="nmx")
            nc.vector.tensor_scalar_mul(nmx[:], mx[:], -1.0)
            ex = wk.tile([8, D], F32, tag=f"ex{c}", bufs=1)
            ssum = wk.tile([8, 1], F32, tag="esum")
            nc.scalar.activation(ex[:], uT_sb[c][:], AF.Exp, bias=nmx[:, :],
                                 accum_out=ssum[:])
            rcp = wk.tile([8, 1], F32, tag=f"ercp{c}", bufs=1)
            nc.vector.reciprocal(rcp[:], ssum[:])
            return ex, rcp          # ex is UNNORMALIZED; rcp folded into las

        def attn_v(c, ex, rcp, fills=False):
            """las[i,:] = (xa_i @ Wq.T + peA_i) segments combined with M2,
            where xa_i = sum_s ex_i[s] x[s,:], all normalized by rcp[i]."""
            # aT: ex [8,512] -> 4x [128,8] bf16
            aTss = []
            for st in range(4):
                atp = ps.tile([128, 8], F32, tag="vc", bufs=4)
                nc.tensor.transpose(atp[:], ex[:, st * 128:(st + 1) * 128],
                                    eye_sb[:8, :8])
                aTs = wk.tile([128, 8], BF16, tag=f"aT{c}{st}", bufs=1)
                nc.vector.tensor_copy(aTs[:], atp[:])
                aTss.append(aTs)
            # xa[i, K] = sum_s a_i[s] x[s, K]   (contract s on the PE)
            xa = ps.tile([8, D], F32, tag="vc", bufs=4)
            for st in range(4):
                nc.tensor.matmul(
                    xa[:], aTss[st][:],
                    xr_all[:, (c * 4 + st) * D:(c * 4 + st + 1) * D],
                    start=(st == 0), stop=(st == 3))
            xas = wk.tile([8, D], F16, tag=f"xas{c}", bufs=1)
            nc.scalar.copy(xas[:], xa[:])
            if fills:
                warm_fill(5)
            # group sums gT[j, i] = sum_{s in group j of tile st} a_i[s]
            gt_ps = ps.tile([8, 32], F32, tag="vc", bufs=4)
            for st in range(4):
                nc.tensor.matmul(gt_ps[:, st * 8:(st + 1) * 8],
                                 selTTb, aTss[st][:], start=True, stop=True)
            gts = wk.tile([8, 32], F16, tag=f"gts{c}", bufs=1)
            nc.vector.tensor_copy(gts[:], gt_ps[:])
            # xaT: 4 transposes [8,128] -> [128,8] f16 into one psum tile
            xat_ps = ps.tile([128, 32], F16, tag="vc", bufs=4)
            for t in range(4):
                nc.tensor.transpose(xat_ps[:, t * 8:(t + 1) * 8],
                                    xas[:, t * 128:(t + 1) * 128], eye8h)
            xat = wk.tile([128, 32], F16, tag=f"xat{c}", bufs=1)
            nc.vector.tensor_copy(xat[:], xat_ps[:])
            # v[i, :] = sum_t xaT_t.T @ wq_t  +  sum_st gT_st.T @ peT_st
            v_ps = ps.tile([8, D], F32, tag="vc", bufs=4)
            for t in range(4):
                nc.tensor.matmul(v_ps[:], xat[:, t * 8:(t + 1) * 8],
                                 wq_all[:, t * D:(t + 1) * D],
                                 start=(t == 0), stop=False)
            for st in range(4):
                m = c * 4 + st
                nc.tensor.matmul(
                    v_ps[:], gts[:, st * 8:(st + 1) * 8],
                    cr8[:, O_PET + m * D:O_PET + (m + 1) * D],
                    start=False, stop=(st == 3))
            vs = wk.tile([8, D], F32, tag=f"vs{c}", bufs=1)
            nc.scalar.copy(vs[:], v_ps[:])
            if fills:
                warm_fill(5)
            # vm[p, 2jt + p//64] = v[2jt + p//64, jt*128+p]  (head segments)
            vm = wk.tile([128, 8], BF16, tag=f"vm{c}", bufs=1)
            nc.vector.memset(vm[:], 0.0)
            for jt in range(4):
                vtp = ps.tile([128, 8], F32, tag="vc", bufs=4)
                nc.tensor.transpose(vtp[:], vs[:, jt * 128:(jt + 1) * 128],
                                    eye_sb[:8, :8])
                nc.vector.tensor_copy(vm[0:64, 2 * jt:2 * jt + 1],
                                      vtp[0:64, 2 * jt:2 * jt + 1])
                nc.vector.tensor_copy(vm[64:128, 2 * jt + 1:2 * jt + 2],
                                      vtp[64:128, 2 * jt + 1:2 * jt + 2])
            lap = ps.tile([8, D], F32, tag="vc", bufs=4)
            nc.tensor.matmul(lap[:], vm[:], Mstcb, start=True, stop=True)
            las = wk.tile([8, D], BF16, tag=f"las{c}", bufs=1)
            nc.vector.tensor_scalar_mul(las[:], lap[:], rcp[:])  # 1/Z here
            return las

        def ln_rsd(z):
            """bn stats -> (mu, rsd = 1/sqrt(var+eps)) from tile/PSUM z."""
            st6 = wk.tile([128, 6], F32, tag="ls")
            nc.vector.bn_stats(st6[:], z[:])
            mv = wk.tile([128, 2], F32, tag="lm")
            nc.vector.bn_aggr(mv[:], st6[:])
            sd = wk.tile([128, 1], F32, tag="lsd")
            nc.scalar.activation(sd[:], mv[:, 1:2], AF.Sqrt, bias=epsT[:, :])
            rsd = wk.tile([128, 1], F32, tag="lr")
            nc.vector.reciprocal(rsd[:], sd[:])
            return mv, rsd

        def attn_resid(c, las, jt):
            m = c * 4 + jt
            bcp = ps.tile([128, D], F32, tag="mmB", bufs=2)
            nc.tensor.matmul(bcp[:],
                             cs8[0:8, O_E8 + jt * 128:O_E8 + (jt + 1) * 128],
                             las[:], start=True, stop=False)
            # z1 += x residual on the PE (keeps the DVE chain short)
            nc.tensor.matmul(bcp[:], EYBcb, xr_all[:, m * D:(m + 1) * D],
                             start=False, stop=True)
            mv, rsd = ln_rsd(bcp)
            nc.vector.tensor_scalar(nrm1[m][:], bcp[:], mv[:, 0:1], rsd[:],
                                    op0=ALU.subtract, op1=ALU.mult)
            if not trivial:
                nc.gpsimd.tensor_mul(o1_sb[m][:], nrm1[m][:], G1cb[:])
                nc.gpsimd.tensor_add(o1_sb[m][:], o1_sb[m][:], B1Tcb[:])
            # o1T via 4 PE transposes into one PSUM bank + a single copy
            tps = ps.tile([128, D], BF16, tag="vc", bufs=4)
            for t in range(4):
                nc.tensor.transpose(tps[:, t * 128:(t + 1) * 128],
                                    nrm1[m][:, t * 128:(t + 1) * 128], EYBcb)
            dst = o1T_all[:, m * D:(m + 1) * D]
            if m % 2 == 0:
                nc.scalar.copy(dst, tps[:])
            else:
                nc.vector.tensor_copy(dst, tps[:])

        h1map = {}

        # strided rhs view: oTr[p, m, t, y] = o1T_all[p, m*512 + t*128 + y]
        oTr = o1T_all[:].rearrange("p (m t y) -> p m t y", m=8, t=4)

        def ffn_h1(h, ft, split=False):
            p1 = ps.tile([128, D], F32, tag="mmA", bufs=2)
            if split:
                # two N=256 halves: the first needs only m-tiles (h*4, h*4+1)
                for half in range(2):
                    for dt in range(4):
                        nc.tensor.matmul(
                            p1[:, half * 256:(half + 1) * 256],
                            w1_all[:, dt * DFF + ft * 128:
                                   dt * DFF + (ft + 1) * 128],
                            oTr[:, h * 4 + 2 * half:h * 4 + 2 * half + 2,
                                dt, :],
                            start=(half == 0 and dt == 0), stop=(dt == 3),
                            skip_group_check=True)
            else:
                for dt in range(4):
                    nc.tensor.matmul(
                        p1[:],
                        w1_all[:, dt * DFF + ft * 128:dt * DFF + (ft + 1) * 128],
                        oTr[:, h * 4:(h + 1) * 4, dt, :],
                        start=(dt == 0), stop=(dt == 3))
            h1t = hp.tile([128, D], BF16, tag=f"h1_{ft}", bufs=2,
                          name=f"h1_{h}_{ft}")
            nc.scalar.activation(h1t[:], p1[:], AF.Relu,
                                 bias=cf[:, O_B1P + ft:O_B1P + ft + 1])
            h1map[(h, ft)] = h1t

        def ffn_rm(m):
            h, rm = divmod(m, 4)
            tail = m >= 6
            p2 = ps.tile([128, D], F32, tag="mmB", bufs=2)
            for ft in range(16):
                nc.tensor.matmul(
                    p2[:], h1map[(h, ft)][:, rm * 128:(rm + 1) * 128],
                    w2_all[:, ft * D:(ft + 1) * D],
                    start=(ft == 0), stop=(ft == 15 and not tail))
            if tail:
                # z2 += o1 on the (tail-idle) PE; LN2 reads PSUM directly
                nc.tensor.matmul(p2[:], EYBcb, o1_sb[m][:],
                                 start=False, stop=True)
                z2 = p2
            else:
                z2t = wk.tile([128, D], BF16, tag="z2")
                nc.vector.tensor_add(z2t[:], p2[:], o1_sb[m][:])
                z2 = z2t
            mv, rsd = ln_rsd(z2)
            if trivial:
                yt = wk.tile([128, D], BF16, tag="yt")
                nc.vector.tensor_scalar(yt[:], z2[:], mv[:, 0:1], rsd[:],
                                        op0=ALU.subtract, op1=ALU.mult)
            else:
                nrm2 = wk.tile([128, D], BF16, tag="n2")
                nc.vector.tensor_scalar(nrm2[:], z2[:], mv[:, 0:1], rsd[:],
                                        op0=ALU.subtract, op1=ALU.mult)
                tg = wk.tile([128, D], BF16, tag="tg")
                yt = wk.tile([128, D], BF16, tag="yt")
                if tail:
                    nc.vector.tensor_mul(tg[:], nrm2[:], G2cb[:])
                    nc.vector.tensor_add(yt[:], tg[:], BE2cb[:])
                else:
                    nc.gpsimd.tensor_mul(tg[:], nrm2[:], G2cb[:])
                    nc.gpsimd.tensor_add(yt[:], tg[:], BE2cb[:])
            nc.sync.dma_start(out[m * 128:(m + 1) * 128, :], yt[:])

        # ---------------- schedule ----------------
        uT_mm(0)
        warm_fill(6)
        uT_mm(1)
        warm_fill(10)
        ex0, rcp0 = attn_softmax(0)
        ex1, rcp1 = attn_softmax(1)
        # preload the sqrt-family table set: reading ex1 (written by the
        # second Exp on this same queue) forces this AFTER both Exps
        junk8 = cst.tile([8, 1], F32, tag="junk8", name="junk8")
        nc.scalar.activation(junk8[:], ex1[:, 0:1], AF.Sqrt)
        w1_load[0]()
        w1_load[1]()
        warm_fill(8)
        las0 = attn_v(0, ex0, rcp0, fills=True)
        w1_load[2]()
        w1_load[3]()
        attn_resid(0, las0, 0)
        attn_resid(0, las0, 1)
        w1_load[4]()
        w1_load[5]()
        attn_resid(0, las0, 2)
        attn_resid(0, las0, 3)
        w1_load[6]()
        w1_load[7]()
        las1 = attn_v(1, ex1, rcp1)
        for ft in range(4):
            ffn_h1(0, ft, split=True)
        attn_resid(1, las1, 0)
        for ft in range(4, 8):
            ffn_h1(0, ft, split=True)
        attn_resid(1, las1, 1)
        for ft in range(8, 12):
            ffn_h1(0, ft, split=True)
        attn_resid(1, las1, 2)
        for ft in range(12, 16):
            ffn_h1(0, ft, split=True)
        attn_resid(1, las1, 3)
        ffn_rm(0)
        for ft in range(0, 4):
            ffn_h1(1, ft)
        ffn_rm(1)
        for ft in range(4, 8):
            ffn_h1(1, ft)
        ffn_rm(2)
        for ft in range(8, 12):
            ffn_h1(1, ft)
        ffn_rm(3)
        for ft in range(12, 16):
            ffn_h1(1, ft)
        for m in range(4, 8):
            ffn_rm(m)

    nc.compile()
    return nc


def _pe_table():
    pos = np.arange(S, dtype=np.float32)[:, None]
    div = np.exp(np.arange(0, D, 2, dtype=np.float32) * (-math.log(10000.0) / D))
    ang = pos * div
    pe = np.zeros((S, D), np.float32)
    pe[:, 0::2] = np.sin(ang)
    pe[:, 1::2] = np.cos(ang)
    return pe


def _is_trivial(W1, b1, b2, g1, be1, g2, be2):
    f32 = lambda a: np.asarray(a, dtype=np.float32)
    return (np.all(f32(g1) == 1) and np.all(f32(be1) == 0)
            and np.all(f32(b2) == 0) and np.all(f32(g2) == 1)
            and np.all(f32(be2) == 0))


def make_in_maps(x, Wq, Wfc, W1, b1, W2, b2, g1, be1, g2, be2):
    f32 = lambda a: np.ascontiguousarray(a, dtype=np.float32)
    bfc = lambda a: np.ascontiguousarray(np.asarray(f32(a), dtype="bfloat16"))
    xf = f32(x).reshape(S * H * W, D)
    pe = _pe_table()
    M2 = f32(Wfc).reshape(D, NH, DEP).sum(axis=1).T          # (64, 512)
    Mstk = np.concatenate([M2, M2], axis=0)                  # (128, 512)

    WqTf = f32(Wq.T)
    Wu = WqTf.reshape(D, NH, DEP).sum(axis=2)                # (K, i)
    WUc = np.ascontiguousarray(
        Wu.reshape(4, 128, NH).transpose(1, 0, 2).reshape(128, 32),
        dtype=np.float16)

    CF = np.zeros((128, NCF), np.float32)
    CF[:, O_EYE:O_EYE + 128] = np.eye(128, dtype=np.float32)
    b1p = f32(b1) + f32(W1) @ f32(be1)                       # be1 folded
    CF[:, O_B1P:O_B1P + 16] = b1p.reshape(16, 128).T
    CF[:, O_WU:O_WU + 16] = WUc.view(np.float32)

    selTT = np.zeros((128, 8), np.float32)                   # [s, j]
    for p in range(128):
        selTT[p, p // 16] = 1.0

    CB = np.zeros((128, NCB), np.float32)
    CB[:, O_EYB:O_EYB + 128] = np.eye(128, dtype=np.float32)
    CB[:, O_MST:O_MST + D] = Mstk
    CB[:, O_SELB:O_SELB + 8] = selTT

    CS = np.zeros((8, NCS), np.float32)
    for jt in range(4):
        for p in range(128):
            CS[2 * jt + p // 64, O_E8 + jt * 128 + p] = 1.0
    CS[0, O_G1R:O_G1R + D] = f32(g1)
    CS[0, O_G2R:O_G2R + D] = f32(g2)
    CS[0, O_BE2R:O_BE2R + D] = f32(be2)
    CS[0, O_B1TR:O_B1TR + D] = f32(b2) + f32(be1)

    shared = dict(
        WqT=np.ascontiguousarray(WqTf, dtype=np.float16),
        W1T=bfc(f32(W1) * f32(g1)[None, :]).T.copy(),        # g1 folded
        W2T=bfc(f32(W2).T),
        CF=CF, CB=bfc(CB), CS8=bfc(CS),
    )
    selT = np.zeros((8, 128), np.float32)
    for rr in range(128):
        selT[rr // 16, rr] = 1.0

    peu_all = pe.reshape(S, NH, DEP).sum(axis=2)             # (S, 8) f32

    maps = []
    for k in range(NCORES):
        sl = xf[k * R:(k + 1) * R]
        m = dict(shared)
        slT = np.asarray(sl.T, dtype=np.float16)
        # xq layout: row-block i = m-pair (2i, 2i+1); columns (dt, mi, c)
        arr = slT.reshape(4, 128, 4, 2, 128)        # (t, p, i, mi, c)
        arr = arr.transpose(2, 1, 0, 3, 4)          # (i, p, t, mi, c)
        m["xT"] = np.ascontiguousarray(arr.reshape(512, 1024))
        # xRb[p, t*512+d] = x[t*128+p, d]
        m["xRb"] = bfc(sl.reshape(8, 128, D).transpose(1, 0, 2)
                       .reshape(128, 8 * D))
        cr8 = np.zeros((8, NCR8), np.float16)
        cr8[:, O_SEL:O_SEL + 128] = selT
        pe_loc = pe[k * 64:(k + 1) * 64]            # (64, 512)
        cr8[:, O_PET:O_PET + 8 * D] = (pe_loc.reshape(8, 8, D)
                                       .transpose(1, 0, 2).reshape(8, 8 * D))
        cr8[:, O_EY8:O_EY8 + 8] = np.eye(8, dtype=np.float16)
        # pe segment sums, exact f32, laid out [i, c*512 + st*128 + y]
        peu_loc = peu_all[k * 64:(k + 1) * 64]      # (64, 8)
        cu = np.zeros((8, 2 * D), np.float32)
        for mm in range(8):
            c, st = divmod(mm, 4)
            blk = peu_loc[mm * 8:(mm + 1) * 8]      # (j, i)
            cu[:, c * D + st * 128:c * D + (st + 1) * 128] = blk.T @ selT
        cr8[:, O_PEU:O_PEU + 4 * D] = cu.view(np.float16)
        m["CR8"] = np.ascontiguousarray(cr8)
        maps.append(m)
    return maps


def kernel(x, Wq, Wfc, W1, b1, W2, b2, g1, be1, g2, be2, _results_hook=None,
           _trace=False, _tmpdir=None):
    trivial = _is_trivial(W1, b1, b2, g1, be1, g2, be2)
    key = ("nc", trivial)
    if key not in _cached:
        _cached[key] = build_nc(trivial)
    nc = _cached[key]
    in_maps = make_in_maps(x, Wq, Wfc, W1, b1, W2, b2, g1, be1, g2, be2)
    res = run_bass_kernel_spmd(nc, in_maps, list(range(NCORES)),
                               trace=_trace, tmpdir=_tmpdir)
    if _results_hook is not None:
        _results_hook(res)
    y = np.concatenate([np.asarray(res.results[k]["out"], dtype=np.float32)
                        for k in range(NCORES)], axis=0)
    return y.reshape(S, H, W, D)


# revision 18
# speedup vs baseline: 1.1940x; 1.1940x over previous
"""Trainium2 Bass kernel for nn_EncoderLayer_73315091743398.

The reference module's attention einsums ('hwink,hwijm->hwinm') sum their k/j
indices independently, so the whole attention block collapses to, per
(h,w)-chunk c and head i, over the flat q matrix qf = x@Wq.T + pe viewed as
(8192, 512) in raw (s,h,w) row order:

    u[s]  = sum_d qf[c*512+s, 64i+d]          (segment row sums)
    a     = softmax_s(u)
    v[d]  = sum_s a[s] * qf[c*512+s, 64i+d]
    row   = tile8(v) @ Wfc.T = v @ M,  M[d,:] = sum_b Wfc[:, 64b+d].T

and attn_out viewed (S,H,W,D) has row A[s'] = row_{c=s'//32, i=(s'%32)//4},
independent of (h,w).  Core k owns raw rows [k*1024,(k+1)*1024): these are
exactly attention chunks {2k, 2k+1} AND the residual/FFN rows for
s' in [64k, 64k+64), so the 8 cores run fully independent SPMD programs.

v3 structure (vs the 120.7us baseline):
  - uT[i,s] is computed DIRECTLY as x @ Wu (Wu = segment-summed Wq.T
    columns) into an [8,512] PSUM per chunk + an exact f32 pe-segment-sum
    added on the DVE; softmax for both chunks runs early and entirely off
    the FFN critical path.
  - The value pass never materializes q: by associativity,
      v_i = (sum_s a_i[s] x[s,:]) @ Wq.T + (sum_s a_i[s] pe[s,:])
    so per chunk: xa = aT.T @ x (4 matmuls), v = xaT @ WqT (4) plus the
    pe term via group-sum selectors (8 tiny).  This deletes the 40 q
    matmuls and the 8 scalar-engine PSUM->SBUF q copies of the q path.
  - 1/sum(exp) is folded into the las row scale (tensor_scalar_mul),
    removing the ex normalization from the a-tensor critical path.
  - Exp and Sqrt ACT table sets would thrash (different sets, ~2.7us per
    switch): both softmax Exps are issued back-to-back, then a junk Sqrt
    that READS the second exp's output (ordering-proof) preloads the
    sqrt set once for all 16 LN chains.
  - o1T for the FFN is built with 4 PE transposes per tile into one PSUM
    bank + a single copy, replacing DMA transposes (1.2-1.5us descriptor
    stalls on the HWDGE queues).
  - Input DMA descriptor generation (~0.6-0.8us each on an HWDGE queue!)
    is split across BOTH queues (sync + scalar), small consts are packed
    into two tensors (bitcast views), and w1/w2 arrive in per-ft chunks
    so the FFN streams against DMA arrival.
  - Trivial LN affine params / biases (true for this problem's inputs)
    drop the GpSimd scale/bias ops; output is bf16, upcast on host.
"""

import math
import os
import sys
from contextlib import ExitStack

import numpy as np
import ml_dtypes  # noqa: F401  (registers bfloat16)

for _p in ("/opt/trn_rl_repo", "/root/.axon_site/_ro/trn_rl_repo"):
    if os.path.isdir(_p) and _p not in sys.path:
        sys.path.append(_p)

import concourse.bass as bass
import concourse.bacc as bacc
import concourse.mybir as mybir
import concourse.tile as tile
from concourse.bass_utils import run_bass_kernel_spmd

F32 = mybir.dt.float32
F16 = mybir.dt.float16
BF16 = mybir.dt.bfloat16
AF = mybir.ActivationFunctionType
ALU = mybir.AluOpType
AX = mybir.AxisListType

S, H, W, D = 512, 4, 4, 512
NH, DEP, DFF = 8, 64, 2048
NCORES = 8
R = 1024          # rows per core of the flat (8192, 512) view
EPS = 1e-5

# CF (f32): eye128 [128], b1p [16], wu-bitcast [16] -> 160
O_EYE, O_B1P, O_WU = 0, 128, 144
NCF = 160
# CB (bf16): eye128 + Mst + selTTb [8]
O_EYB, O_MST, O_SELB = 0, 128, 640
NCB = 648
# CS8 (bf16, 8 partitions): E8 selector rows + rows of g1/g2/be2/(b2+be1)
O_E8, O_G1R, O_G2R, O_BE2R, O_B1TR = 0, 512, 1024, 1536, 2048
NCS = 2560
# CR8 (fp16, 8 partitions): selT [8,128], peT [8,8*512], eye8 [8,8],
# peuT-bitcast [8, 2*512 f32]
O_SEL, O_PET, O_EY8, O_PEU = 0, 128, 128 + 8 * D, 128 + 8 * D + 8
NCR8 = O_PEU + 4 * D          # 2*512 f32 as 4*512 f16 halves

_cached = {}


def build_nc(trivial):
    """Single-core SPMD Bass/Tile program (same program on all 8 cores).

    trivial=True: g1==1, be1==0, b2==0, g2==1, be2==0 (the actual inputs);
    drops the residual scale and the output scale/bias ops.
    """
    nc = bacc.Bacc("TRN2", debug=False, target_bir_lowering=False)

    xT = nc.dram_tensor("xT", [D, R], F16, kind="ExternalInput")
    xRb = nc.dram_tensor("xRb", [128, 8 * D], BF16, kind="ExternalInput")
    WqT = nc.dram_tensor("WqT", [D, D], F16, kind="ExternalInput")
    W1T = nc.dram_tensor("W1T", [D, DFF], BF16, kind="ExternalInput")
    W2T = nc.dram_tensor("W2T", [DFF, D], BF16, kind="ExternalInput")
    CF = nc.dram_tensor("CF", [128, NCF], F32, kind="ExternalInput")
    CB = nc.dram_tensor("CB", [128, NCB], BF16, kind="ExternalInput")
    CS8 = nc.dram_tensor("CS8", [8, NCS], BF16, kind="ExternalInput")
    CR8 = nc.dram_tensor("CR8", [8, NCR8], F16, kind="ExternalInput")
    out = nc.dram_tensor("out", [R, D], BF16, kind="ExternalOutput")

    with ExitStack() as ctx:
        tc = ctx.enter_context(tile.TileContext(nc))
        cst = ctx.enter_context(tc.tile_pool(name="cst", bufs=1))
        xp = ctx.enter_context(tc.tile_pool(name="xp", bufs=1))
        qp = ctx.enter_context(tc.tile_pool(name="qp", bufs=1))
        hp = ctx.enter_context(tc.tile_pool(name="hp", bufs=1))
        wk = ctx.enter_context(tc.tile_pool(name="wk", bufs=2))
        ps = ctx.enter_context(tc.tile_pool(name="ps", bufs=1, space="PSUM"))

        # ---- loads, split across BOTH HWDGE queues (descriptor generation
        #      is ~0.6-0.8us each and would serialize on one queue).
        # sync queue: cr8(+pe sums), xq0..3, xr halves, w2 chunks
        # scalar queue: cf(+wu), wq, cb, cs8, w1 chunks
        cr8 = cst.tile([8, NCR8], F16, tag="cr8", name="cr8")
        nc.sync.dma_start(cr8[:], CR8[:])
        cf = cst.tile([128, NCF], F32, tag="cf", name="cf")
        nc.scalar.dma_start(cf[:], CF[:])
        xq = [xp.tile([128, R], F16, tag=f"dT{i}", name=f"xq{i}")
              for i in range(4)]
        for i in range(4):
            nc.sync.dma_start(xq[i][:], xT[i * 128:(i + 1) * 128, :])
        wq_all = cst.tile([128, 4 * D], F16, tag="wq", name="wq_all")
        nc.scalar.dma_start(
            wq_all[:].rearrange("p (t j) -> p t j", t=4),
            WqT.rearrange("(t p) j -> p t j", p=128))
        cb = cst.tile([128, NCB], BF16, tag="cb", name="cb")
        nc.scalar.dma_start(cb[:], CB[:])
        cs8 = cst.tile([8, NCS], BF16, tag="cs8", name="cs8")
        nc.scalar.dma_start(cs8[:], CS8[:])
        xr_all = xp.tile([128, 8 * D], BF16, tag="xr", name="xr_all")
        nc.sync.dma_start(xr_all[:, 0:4 * D], xRb[:, 0:4 * D])
        nc.sync.dma_start(xr_all[:, 4 * D:], xRb[:, 4 * D:])
        # w1 in 2-ft chunks; descriptors are ISSUED LATER (closures invoked
        # at schedule points) so they don't block the softmax ACTs on the
        # scalar HWDGE queue
        w1_all = cst.tile([128, 4 * DFF], BF16, tag="w1", name="w1_all")
        w1v = w1_all[:].rearrange("p (t f j) -> p t f j", t=4, f=8)
        w1s = W1T.rearrange("(t p) (f j) -> p t f j", p=128, f=8)
        w1_load = [lambda ft=ft: nc.scalar.dma_start(
            w1v[:, :, ft, :], w1s[:, :, ft, :]) for ft in range(8)]
        # w2 in 4-ft chunks on the sync queue (needed last)
        w2_all = cst.tile([128, 16 * D], BF16, tag="w2", name="w2_all")
        w2v = w2_all[:].rearrange("p (f d) -> p f d", f=16)
        w2s = W2T.rearrange("(f p) d -> p f d", p=128)
        for ft in range(0, 16, 4):
            nc.sync.dma_start(w2v[:, ft:ft + 4, :], w2s[:, ft:ft + 4, :])

        eye_sb = cf[:, O_EYE:O_EYE + 128]
        wu_sb = cf[:, O_WU:O_WU + 16].bitcast(F16)        # [128, 32] f16
        cu8 = cr8[:, O_PEU:O_PEU + 4 * D].bitcast(F32)    # [8, 1024] f32
        eye8h = cr8[0:8, O_EY8:O_EY8 + 8]                 # [8, 8] f16
        Mstcb = cb[:, O_MST:O_MST + D]
        EYBcb = cb[:, O_EYB:O_EYB + 128]
        selTTb = cb[:, O_SELB:O_SELB + 8]                 # [128, 8] bf16
        epsT = cst.tile([128, 1], F32, tag="eps", name="epsT")
        nc.vector.memset(epsT[:], EPS)

        if not trivial:
            G1cb = cst.tile([128, D], BF16, tag="g1t", name="g1t")
            G2cb = cst.tile([128, D], BF16, tag="g2t", name="g2t")
            BE2cb = cst.tile([128, D], BF16, tag="be2t", name="be2t")
            B1Tcb = cst.tile([128, D], BF16, tag="b1tt", name="b1tt")
            for bt, off in ((G1cb, O_G1R), (G2cb, O_G2R),
                            (BE2cb, O_BE2R), (B1Tcb, O_B1TR)):
                nc.gpsimd.partition_broadcast(bt[:], cs8[0:1, off:off + D])

        # ---- ACT table preload (exp set) + PE warm-up during DMA wait.
        # N=512 warm matmuls: only a sustained wide stream flips HAM to
        # 8/8 (N=64 junk measurably never does); later warm_fill() calls
        # bridge dependency stalls in the attention phase so the clock
        # never re-throttles before the FFN stream takes over.
        junk = cst.tile([128, 1], F32, tag="junk", name="junk")
        nc.scalar.activation(junk[:], epsT[:], AF.Exp)
        warm_sb = cst.tile([128, D], BF16, tag="wrm", name="warm_sb")
        nc.vector.memset(warm_sb[:], 0.0)

        def warm_fill(n, cols=256):
            wt = ps.tile([128, D], F32, tag="wrm", bufs=1)
            for _ in range(n):
                nc.tensor.matmul(wt[:, 0:cols], warm_sb[:, 0:128],
                                 warm_sb[:, 0:cols], start=True, stop=True)

        warm_fill(10, cols=512)

        nrm1 = [qp.tile([128, D], BF16, tag=f"n1{m}", name=f"nrm1_{m}")
                for m in range(8)]
        if not trivial:
            o1_sb = [qp.tile([128, D], BF16, tag=f"o1{m}", name=f"o1sb{m}")
                     for m in range(8)]
        else:
            o1_sb = nrm1
        # o1T_all[p, m*512 + t*128 + y] = nrm1[m][y, t*128 + p]
        o1T_all = qp.tile([128, 8 * D], BF16, tag="oT", name="o1T_all")

        uT_sb = [qp.tile([8, D], F32, tag=f"uT{c}", name=f"uTsb{c}")
                 for c in range(2)]

        def uT_mm(c):
            """uT_ps[c][i, st*128+y] = sum_K x[row, K] * Wu[K, i] for the
            four tiles st of chunk c (cols 256*j from xq[2c+j])."""
            ups = ps.tile([8, D], F32, tag="mmA", bufs=2)
            # j=1 range: start=False on untouched PSUM (has_written unset
            # -> overwrite), so j=0's accumulating values aren't clobbered
            for j in range(2):
                for t in range(4):
                    nc.tensor.matmul(
                        ups[:, j * 256:(j + 1) * 256],
                        wu_sb[:, t * 8:(t + 1) * 8],
                        xq[2 * c + j][:, t * 256:(t + 1) * 256],
                        start=(j == 0 and t == 0), stop=(t == 3),
                        skip_group_check=True)
            # exact pe segment sums added on DVE (f16 would cost ~0.02 abs)
            nc.vector.tensor_add(uT_sb[c][:], ups[:],
                                 cu8[:, c * D:(c + 1) * D])

        def attn_softmax(c):
            mx = wk.tile([8, 1], F32, tag="mx")
            nc.vector.tensor_reduce(mx[:], uT_sb[c][:], axis=AX.X, op=ALU.max)
            nmx = wk.tile([8, 1], F32, tag="nmx")
            nc.vector.tensor_scalar_mul(nmx[:], mx[:], -1.0)
            ex = wk.tile([8, D], F32, tag=f"ex{c}", bufs=1)
            ssum = wk.tile([8, 1], F32, tag="esum")
            nc.scalar.activation(ex[:], uT_sb[c][:], AF.Exp, bias=nmx[:, :],
                                 accum_out=ssum[:])
            rcp = wk.tile([8, 1], F32, tag=f"ercp{c}", bufs=1)
            nc.vector.reciprocal(rcp[:], ssum[:])
            return ex, rcp          # ex is UNNORMALIZED; rcp folded into las

        def attn_v(c, ex, rcp, fills=False):
            """las[i,:] = (xa_i @ Wq.T + peA_i) segments combined with M2,
            where xa_i = sum_s ex_i[s] x[s,:], all normalized by rcp[i]."""
            # aT: ex [8,512] -> 4x [128,8] bf16
            aTss = []
            for st in range(4):
                atp = ps.tile([128, 8], F32, tag="vc", bufs=3)
                nc.tensor.transpose(atp[:], ex[:, st * 128:(st + 1) * 128],
                                    eye_sb[:8, :8])
                aTs = wk.tile([128, 8], BF16, tag=f"aT{c}{st}", bufs=1)
                nc.vector.tensor_copy(aTs[:], atp[:])
                aTss.append(aTs)
            # xa[i, K] = sum_s a_i[s] x[s, K]   (contract s on the PE)
            xa = ps.tile([8, D], F32, tag="vc", bufs=3)
            for st in range(4):
                nc.tensor.matmul(
                    xa[:], aTss[st][:],
                    xr_all[:, (c * 4 + st) * D:(c * 4 + st + 1) * D],
                    start=(st == 0), stop=(st == 3))
            xas = wk.tile([8, D], F16, tag=f"xas{c}", bufs=1)
            nc.scalar.copy(xas[:], xa[:])
            if fills:
                warm_fill(5)
            # group sums gT[j, i] = sum_{s in group j of tile st} a_i[s]
            gt_ps = ps.tile([8, 32], F32, tag="vc", bufs=3)
            for st in range(4):
                nc.tensor.matmul(gt_ps[:, st * 8:(st + 1) * 8],
                                 selTTb, aTss[st][:], start=True, stop=True)
            gts = wk.tile([8, 32], F16, tag=f"gts{c}", bufs=1)
            nc.vector.tensor_copy(gts[:], gt_ps[:])
            # xaT: 4 transposes [8,128] -> [128,8] f16 into one psum tile
            xat_ps = ps.tile([128, 32], F16, tag="vc", bufs=3)
            for t in range(4):
                nc.tensor.transpose(xat_ps[:, t * 8:(t + 1) * 8],
                                    xas[:, t * 128:(t + 1) * 128], eye8h)
            xat = wk.tile([128, 32], F16, tag=f"xat{c}", bufs=1)
            nc.vector.tensor_copy(xat[:], xat_ps[:])
            # v[i, :] = sum_t xaT_t.T @ wq_t  +  sum_st gT_st.T @ peT_st
            v_ps = ps.tile([8, D], F32, tag="vc", bufs=3)
            for t in range(4):
                nc.tensor.matmul(v_ps[:], xat[:, t * 8:(t + 1) * 8],
                                 wq_all[:, t * D:(t + 1) * D],
                                 start=(t == 0), stop=False)
            for st in range(4):
                m = c * 4 + st
                nc.tensor.matmul(
                    v_ps[:], gts[:, st * 8:(st + 1) * 8],
                    cr8[:, O_PET + m * D:O_PET + (m + 1) * D],
                    start=False, stop=(st == 3))
            vs = wk.tile([8, D], F32, tag=f"vs{c}", bufs=1)
            nc.scalar.copy(vs[:], v_ps[:])
            if fills:
                warm_fill(5)
            # vm[p, 2jt + p//64] = v[2jt + p//64, jt*128+p]  (head segments)
            vm = wk.tile([128, 8], BF16, tag=f"vm{c}", bufs=1)
            nc.vector.memset(vm[:], 0.0)
            for jt in range(4):
                vtp = ps.tile([128, 8], F32, tag="vc", bufs=3)
                nc.tensor.transpose(vtp[:], vs[:, jt * 128:(jt + 1) * 128],
                                    eye_sb[:8, :8])
                nc.vector.tensor_copy(vm[0:64, 2 * jt:2 * jt + 1],
                                      vtp[0:64, 2 * jt:2 * jt + 1])
                nc.vector.tensor_copy(vm[64:128, 2 * jt + 1:2 * jt + 2],
                                      vtp[64:128, 2 * jt + 1:2 * jt + 2])
            lap = ps.tile([8, D], F32, tag="vc", bufs=3)
            nc.tensor.matmul(lap[:], vm[:], Mstcb, start=True, stop=True)
            las = wk.tile([8, D], BF16, tag=f"las{c}", bufs=1)
            nc.vector.tensor_scalar_mul(las[:], lap[:], rcp[:])  # 1/Z here
            return las

        def ln_rsd(z):
            """bn stats -> (mu, rsd = 1/sqrt(var+eps)) from tile/PSUM z."""
            st6 = wk.tile([128, 6], F32, tag="ls")
            nc.vector.bn_stats(st6[:], z[:])
            mv = wk.tile([128, 2], F32, tag="lm")
            nc.vector.bn_aggr(mv[:], st6[:])
            sd = wk.tile([128, 1], F32, tag="lsd")
            nc.scalar.activation(sd[:], mv[:, 1:2], AF.Sqrt, bias=epsT[:, :])
            rsd = wk.tile([128, 1], F32, tag="lr")
            nc.vector.reciprocal(rsd[:], sd[:])
            return mv, rsd

        def attn_resid(c, las, jt):
            m = c * 4 + jt
            bcp = ps.tile([128, D], F32, tag="mmB", bufs=2)
            nc.tensor.matmul(bcp[:],
                             cs8[0:8, O_E8 + jt * 128:O_E8 + (jt + 1) * 128],
                             las[:], start=True, stop=False)
            # z1 += x residual on the PE (keeps the DVE chain short)
            nc.tensor.matmul(bcp[:], EYBcb, xr_all[:, m * D:(m + 1) * D],
                             start=False, stop=True)
            mv, rsd = ln_rsd(bcp)
            nc.vector.tensor_scalar(nrm1[m][:], bcp[:], mv[:, 0:1], rsd[:],
                                    op0=ALU.subtract, op1=ALU.mult)
            if not trivial:
                nc.gpsimd.tensor_mul(o1_sb[m][:], nrm1[m][:], G1cb[:])
                nc.gpsimd.tensor_add(o1_sb[m][:], o1_sb[m][:], B1Tcb[:])
            # o1T via 4 PE transposes into one PSUM bank + a single copy
            tps = ps.tile([128, D], BF16, tag="vc", bufs=3)
            for t in range(4):
                nc.tensor.transpose(tps[:, t * 128:(t + 1) * 128],
                                    nrm1[m][:, t * 128:(t + 1) * 128], EYBcb)
            dst = o1T_all[:, m * D:(m + 1) * D]
            if m % 2 == 0:
                nc.scalar.copy(dst, tps[:])
            else:
                nc.vector.tensor_copy(dst, tps[:])

        h1map = {}

        # strided rhs view: oTr[p, m, t, y] = o1T_all[p, m*512 + t*128 + y]
        oTr = o1T_all[:].rearrange("p (m t y) -> p m t y", m=8, t=4)

        def ffn_h1(h, ft, split=False):
            p1 = ps.tile([128, D], F32, tag="mmA", bufs=2)
            if split:
                # two N=256 halves: the first needs only m-tiles (h*4, h*4+1)
                for half in range(2):
                    for dt in range(4):
                        nc.tensor.matmul(
                            p1[:, half * 256:(half + 1) * 256],
                            w1_all[:, dt * DFF + ft * 128:
                                   dt * DFF + (ft + 1) * 128],
                            oTr[:, h * 4 + 2 * half:h * 4 + 2 * half + 2,
                                dt, :],
                            start=(half == 0 and dt == 0), stop=(dt == 3),
                            skip_group_check=True)
            else:
                for dt in range(4):
                    nc.tensor.matmul(
                        p1[:],
                        w1_all[:, dt * DFF + ft * 128:dt * DFF + (ft + 1) * 128],
                        oTr[:, h * 4:(h + 1) * 4, dt, :],
                        start=(dt == 0), stop=(dt == 3))
            h1t = hp.tile([128, D], BF16, tag=f"h1_{ft}", bufs=2,
                          name=f"h1_{h}_{ft}")
            nc.scalar.activation(h1t[:], p1[:], AF.Relu,
                                 bias=cf[:, O_B1P + ft:O_B1P + ft + 1])
            h1map[(h, ft)] = h1t

        def ffn_rm(m):
            h, rm = divmod(m, 4)
            tail = m >= 6
            p2 = ps.tile([128, D], F32, tag="mmB", bufs=2)
            for ft in range(16):
                nc.tensor.matmul(
                    p2[:], h1map[(h, ft)][:, rm * 128:(rm + 1) * 128],
                    w2_all[:, ft * D:(ft + 1) * D],
                    start=(ft == 0), stop=(ft == 15 and not tail))
            if tail:
                # z2 += o1 on the (tail-idle) PE; LN2 reads PSUM directly
                nc.tensor.matmul(p2[:], EYBcb, o1_sb[m][:],
                                 start=False, stop=True)
                z2 = p2
            else:
                z2t = wk.tile([128, D], BF16, tag="z2")
                nc.vector.tensor_add(z2t[:], p2[:], o1_sb[m][:])
                z2 = z2t
            mv, rsd = ln_rsd(z2)
            if trivial:
                yt = wk.tile([128, D], BF16, tag="yt")
                nc.vector.tensor_scalar(yt[:], z2[:], mv[:, 0:1], rsd[:],
                                        op0=ALU.subtract, op1=ALU.mult)
            else:
                nrm2 = wk.tile([128, D], BF16, tag="n2")
                nc.vector.tensor_scalar(nrm2[:], z2[:], mv[:, 0:1], rsd[:],
                                        op0=ALU.subtract, op1=ALU.mult)
                tg = wk.tile([128, D], BF16, tag="tg")
                yt = wk.tile([128, D], BF16, tag="yt")
                if tail:
                    nc.vector.tensor_mul(tg[:], nrm2[:], G2cb[:])
                    nc.vector.tensor_add(yt[:], tg[:], BE2cb[:])
                else:
                    nc.gpsimd.tensor_mul(tg[:], nrm2[:], G2cb[:])
                    nc.gpsimd.tensor_add(yt[:], tg[:], BE2cb[:])
            nc.sync.dma_start(out[m * 128:(m + 1) * 128, :], yt[:])

        # ---------------- schedule ----------------
        uT_mm(0)
        warm_fill(6)
        uT_mm(1)
        warm_fill(10)
        ex0, rcp0 = attn_softmax(0)
        ex1, rcp1 = attn_softmax(1)
        # preload the sqrt-family table set: reading ex1 (written by the
        # second Exp on this same queue) forces this AFTER both Exps
        junk8 = cst.tile([8, 1], F32, tag="junk8", name="junk8")
        nc.scalar.activation(junk8[:], ex1[:, 0:1], AF.Sqrt)
        w1_load[0]()
        w1_load[1]()
        warm_fill(8)
        las0 = attn_v(0, ex0, rcp0, fills=True)
        w1_load[2]()
        w1_load[3]()
        attn_resid(0, las0, 0)
        attn_resid(0, las0, 1)
        w1_load[4]()
        w1_load[5]()
        attn_resid(0, las0, 2)
        attn_resid(0, las0, 3)
        w1_load[6]()
        w1_load[7]()
        las1 = attn_v(1, ex1, rcp1)
        for ft in range(4):
            ffn_h1(0, ft, split=True)
        attn_resid(1, las1, 0)
        for ft in range(4, 8):
            ffn_h1(0, ft, split=True)
        attn_resid(1, las1, 1)
        for ft in range(8, 12):
            ffn_h1(0, ft, split=True)
        attn_resid(1, las1, 2)
        for ft in range(12, 16):
            ffn_h1(0, ft, split=True)
        attn_resid(1, las1, 3)
        ffn_rm(0)
        for ft in range(0, 4):
            ffn_h1(1, ft)
        ffn_rm(1)
        for ft in range(4, 8):
            ffn_h1(1, ft)
        ffn_rm(2)
        for ft in range(8, 12):
            ffn_h1(1, ft)
        ffn_rm(3)
        for ft in range(12, 16):
            ffn_h1(1, ft)
        for m in range(4, 8):
            ffn_rm(m)

    nc.compile()
    return nc


def _pe_table():
    pos = np.arange(S, dtype=np.float32)[:, None]
    div = np.exp(np.arange(0, D, 2, dtype=np.float32) * (-math.log(10000.0) / D))
    ang = pos * div
    pe = np.zeros((S, D), np.float32)
    pe[:, 0::2] = np.sin(ang)
    pe[:, 1::2] = np.cos(ang)
    return pe


def _is_trivial(W1, b1, b2, g1, be1, g2, be2):
    f32 = lambda a: np.asarray(a, dtype=np.float32)
    return (np.all(f32(g1) == 1) and np.all(f32(be1) == 0)
            and np.all(f32(b2) == 0) and np.all(f32(g2) == 1)
            and np.all(f32(be2) == 0))


def make_in_maps(x, Wq, Wfc, W1, b1, W2, b2, g1, be1, g2, be2):
    f32 = lambda a: np.ascontiguousarray(a, dtype=np.float32)
    bfc = lambda a: np.ascontiguousarray(np.asarray(f32(a), dtype="bfloat16"))
    xf = f32(x).reshape(S * H * W, D)
    pe = _pe_table()
    M2 = f32(Wfc).reshape(D, NH, DEP).sum(axis=1).T          # (64, 512)
    Mstk = np.concatenate([M2, M2], axis=0)                  # (128, 512)

    WqTf = f32(Wq.T)
    Wu = WqTf.reshape(D, NH, DEP).sum(axis=2)                # (K, i)
    WUc = np.ascontiguousarray(
        Wu.reshape(4, 128, NH).transpose(1, 0, 2).reshape(128, 32),
        dtype=np.float16)

    CF = np.zeros((128, NCF), np.float32)
    CF[:, O_EYE:O_EYE + 128] = np.eye(128, dtype=np.float32)
    b1p = f32(b1) + f32(W1) @ f32(be1)                       # be1 folded
    CF[:, O_B1P:O_B1P + 16] = b1p.reshape(16, 128).T
    CF[:, O_WU:O_WU + 16] = WUc.view(np.float32)

    selTT = np.zeros((128, 8), np.float32)                   # [s, j]
    for p in range(128):
        selTT[p, p // 16] = 1.0

    CB = np.zeros((128, NCB), np.float32)
    CB[:, O_EYB:O_EYB + 128] = np.eye(128, dtype=np.float32)
    CB[:, O_MST:O_MST + D] = Mstk
    CB[:, O_SELB:O_SELB + 8] = selTT

    CS = np.zeros((8, NCS), np.float32)
    for jt in range(4):
        for p in range(128):
            CS[2 * jt + p // 64, O_E8 + jt * 128 + p] = 1.0
    CS[0, O_G1R:O_G1R + D] = f32(g1)
    CS[0, O_G2R:O_G2R + D] = f32(g2)
    CS[0, O_BE2R:O_BE2R + D] = f32(be2)
    CS[0, O_B1TR:O_B1TR + D] = f32(b2) + f32(be1)

    shared = dict(
        WqT=np.ascontiguousarray(WqTf, dtype=np.float16),
        W1T=bfc(f32(W1) * f32(g1)[None, :]).T.copy(),        # g1 folded
        W2T=bfc(f32(W2).T),
        CF=CF, CB=bfc(CB), CS8=bfc(CS),
    )
    selT = np.zeros((8, 128), np.float32)
    for rr in range(128):
        selT[rr // 16, rr] = 1.0

    peu_all = pe.reshape(S, NH, DEP).sum(axis=2)             # (S, 8) f32

    maps = []
    for k in range(NCORES):
        sl = xf[k * R:(k + 1) * R]
        m = dict(shared)
        slT = np.asarray(sl.T, dtype=np.float16)
        # xq layout: row-block i = m-pair (2i, 2i+1); columns (dt, mi, c)
        arr = slT.reshape(4, 128, 4, 2, 128)        # (t, p, i, mi, c)
        arr = arr.transpose(2, 1, 0, 3, 4)          # (i, p, t, mi, c)
        m["xT"] = np.ascontiguousarray(arr.reshape(512, 1024))
        # xRb[p, t*512+d] = x[t*128+p, d]
        m["xRb"] = bfc(sl.reshape(8, 128, D).transpose(1, 0, 2)
                       .reshape(128, 8 * D))
        cr8 = np.zeros((8, NCR8), np.float16)
        cr8[:, O_SEL:O_SEL + 128] = selT
        pe_loc = pe[k * 64:(k + 1) * 64]            # (64, 512)
        cr8[:, O_PET:O_PET + 8 * D] = (pe_loc.reshape(8, 8, D)
                                       .transpose(1, 0, 2).reshape(8, 8 * D))
        cr8[:, O_EY8:O_EY8 + 8] = np.eye(8, dtype=np.float16)
        # pe segment sums, exact f32, laid out [i, c*512 + st*128 + y]
        peu_loc = peu_all[k * 64:(k + 1) * 64]      # (64, 8)
        cu = np.zeros((8, 2 * D), np.float32)
        for mm in range(8):
            c, st = divmod(mm, 4)
            blk = peu_loc[mm * 8:(mm + 1) * 8]      # (j, i)
            cu[:, c * D + st * 128:c * D + (st + 1) * 128] = blk.T @ selT
        cr8[:, O_PEU:O_PEU + 4 * D] = cu.view(np.float16)
        m["CR8"] = np.ascontiguousarray(cr8)
        maps.append(m)
    return maps


def kernel(x, Wq, Wfc, W1, b1, W2, b2, g1, be1, g2, be2, _results_hook=None,
           _trace=False, _tmpdir=None):
    trivial = _is_trivial(W1, b1, b2, g1, be1, g2, be2)
    key = ("nc", trivial)
    if key not in _cached:
        _cached[key] = build_nc(trivial)
    nc = _cached[key]
    in_maps = make_in_maps(x, Wq, Wfc, W1, b1, W2, b2, g1, be1, g2, be2)
    res = run_bass_kernel_spmd(nc, in_maps, list(range(NCORES)),
                               trace=_trace, tmpdir=_tmpdir)
    if _results_hook is not None:
        _results_hook(res)
    y = np.concatenate([np.asarray(res.results[k]["out"], dtype=np.float32)
                        for k in range(NCORES)], axis=0)
    return y.reshape(S, H, W, D)


# revision 20
# speedup vs baseline: 1.1944x; 1.0004x over previous
"""Trainium2 Bass kernel for nn_EncoderLayer_73315091743398.

The reference module's attention einsums ('hwink,hwijm->hwinm') sum their k/j
indices independently, so the whole attention block collapses to, per
(h,w)-chunk c and head i, over the flat q matrix qf = x@Wq.T + pe viewed as
(8192, 512) in raw (s,h,w) row order:

    u[s]  = sum_d qf[c*512+s, 64i+d]          (segment row sums)
    a     = softmax_s(u)
    v[d]  = sum_s a[s] * qf[c*512+s, 64i+d]
    row   = tile8(v) @ Wfc.T = v @ M,  M[d,:] = sum_b Wfc[:, 64b+d].T

and attn_out viewed (S,H,W,D) has row A[s'] = row_{c=s'//32, i=(s'%32)//4},
independent of (h,w).  Core k owns raw rows [k*1024,(k+1)*1024): these are
exactly attention chunks {2k, 2k+1} AND the residual/FFN rows for
s' in [64k, 64k+64), so the 8 cores run fully independent SPMD programs.

v3 structure (vs the 120.7us baseline):
  - uT[i,s] is computed DIRECTLY as x @ Wu (Wu = segment-summed Wq.T
    columns) into an [8,512] PSUM per chunk + an exact f32 pe-segment-sum
    added on the DVE; softmax for both chunks runs early and entirely off
    the FFN critical path.
  - The value pass never materializes q: by associativity,
      v_i = (sum_s a_i[s] x[s,:]) @ Wq.T + (sum_s a_i[s] pe[s,:])
    so per chunk: xa = aT.T @ x (4 matmuls), v = xaT @ WqT (4) plus the
    pe term via group-sum selectors (8 tiny).  This deletes the 40 q
    matmuls and the 8 scalar-engine PSUM->SBUF q copies of the q path.
  - 1/sum(exp) is folded into the las row scale (tensor_scalar_mul),
    removing the ex normalization from the a-tensor critical path.
  - Exp and Sqrt ACT table sets would thrash (different sets, ~2.7us per
    switch): both softmax Exps are issued back-to-back, then a junk Sqrt
    that READS the second exp's output (ordering-proof) preloads the
    sqrt set once for all 16 LN chains.
  - o1T for the FFN is built with 4 PE transposes per tile into one PSUM
    bank + a single copy, replacing DMA transposes (1.2-1.5us descriptor
    stalls on the HWDGE queues).
  - Input DMA descriptor generation (~0.6-0.8us each on an HWDGE queue!)
    is split across BOTH queues (sync + scalar), small consts are packed
    into two tensors (bitcast views), and w1/w2 arrive in per-ft chunks
    so the FFN streams against DMA arrival.
  - Trivial LN affine params / biases (true for this problem's inputs)
    drop the GpSimd scale/bias ops; output is bf16, upcast on host.
"""

import math
import os
import sys
from contextlib import ExitStack

import numpy as np
import ml_dtypes  # noqa: F401  (registers bfloat16)

for _p in ("/opt/trn_rl_repo", "/root/.axon_site/_ro/trn_rl_repo"):
    if os.path.isdir(_p) and _p not in sys.path:
        sys.path.append(_p)

import concourse.bass as bass
import concourse.bacc as bacc
import concourse.mybir as mybir
import concourse.tile as tile
from concourse.bass_utils import run_bass_kernel_spmd

F32 = mybir.dt.float32
F16 = mybir.dt.float16
BF16 = mybir.dt.bfloat16
AF = mybir.ActivationFunctionType
ALU = mybir.AluOpType
AX = mybir.AxisListType

S, H, W, D = 512, 4, 4, 512
NH, DEP, DFF = 8, 64, 2048
NCORES = 8
R = 1024          # rows per core of the flat (8192, 512) view
EPS = 1e-5

# CF (f32): eye128 [128], b1p [16], wu-bitcast [16] -> 160
O_EYE, O_B1P, O_WU = 0, 128, 144
NCF = 160
# CB (bf16): eye128 + Mst + selTTb [8]
O_EYB, O_MST, O_SELB = 0, 128, 640
NCB = 648
# CS8 (bf16, 8 partitions): E8 selector rows + rows of g1/g2/be2/(b2+be1)
O_E8, O_G1R, O_G2R, O_BE2R, O_B1TR = 0, 512, 1024, 1536, 2048
NCS = 2560
# CR8 (fp16, 8 partitions): selT [8,128], peT [8,8*512], eye8 [8,8],
# peuT-bitcast [8, 2*512 f32]
O_SEL, O_PET, O_EY8, O_PEU = 0, 128, 128 + 8 * D, 128 + 8 * D + 8
NCR8 = O_PEU + 4 * D          # 2*512 f32 as 4*512 f16 halves

_cached = {}


def build_nc(trivial):
    """Single-core SPMD Bass/Tile program (same program on all 8 cores).

    trivial=True: g1==1, be1==0, b2==0, g2==1, be2==0 (the actual inputs);
    drops the residual scale and the output scale/bias ops.
    """
    nc = bacc.Bacc("TRN2", debug=False, target_bir_lowering=False)

    xT = nc.dram_tensor("xT", [D, R], F16, kind="ExternalInput")
    xRb = nc.dram_tensor("xRb", [128, 8 * D], BF16, kind="ExternalInput")
    WqT = nc.dram_tensor("WqT", [D, D], F16, kind="ExternalInput")
    W1T = nc.dram_tensor("W1T", [D, DFF], BF16, kind="ExternalInput")
    W2T = nc.dram_tensor("W2T", [DFF, D], BF16, kind="ExternalInput")
    CF = nc.dram_tensor("CF", [128, NCF], F32, kind="ExternalInput")
    CB = nc.dram_tensor("CB", [128, NCB], BF16, kind="ExternalInput")
    CS8 = nc.dram_tensor("CS8", [8, NCS], BF16, kind="ExternalInput")
    CR8 = nc.dram_tensor("CR8", [8, NCR8], F16, kind="ExternalInput")
    out = nc.dram_tensor("out", [R, D], BF16, kind="ExternalOutput")

    with ExitStack() as ctx:
        tc = ctx.enter_context(tile.TileContext(nc))
        cst = ctx.enter_context(tc.tile_pool(name="cst", bufs=1))
        xp = ctx.enter_context(tc.tile_pool(name="xp", bufs=1))
        qp = ctx.enter_context(tc.tile_pool(name="qp", bufs=1))
        hp = ctx.enter_context(tc.tile_pool(name="hp", bufs=1))
        wk = ctx.enter_context(tc.tile_pool(name="wk", bufs=2))
        ps = ctx.enter_context(tc.tile_pool(name="ps", bufs=1, space="PSUM"))

        # ---- loads, split across BOTH HWDGE queues (descriptor generation
        #      is ~0.6-0.8us each and would serialize on one queue).
        # sync queue: cr8(+pe sums), xq0..3, xr halves, w2 chunks
        # scalar queue: cf(+wu), wq, cb, cs8, w1 chunks
        cr8 = cst.tile([8, NCR8], F16, tag="cr8", name="cr8")
        nc.sync.dma_start(cr8[:], CR8[:])
        cf = cst.tile([128, NCF], F32, tag="cf", name="cf")
        nc.scalar.dma_start(cf[:], CF[:])
        xq = [xp.tile([128, R], F16, tag=f"dT{i}", name=f"xq{i}")
              for i in range(4)]
        for i in range(4):
            nc.sync.dma_start(xq[i][:], xT[i * 128:(i + 1) * 128, :])
        wq_all = cst.tile([128, 4 * D], F16, tag="wq", name="wq_all")
        nc.sync.dma_start(
            wq_all[:].rearrange("p (t j) -> p t j", t=4),
            WqT.rearrange("(t p) j -> p t j", p=128))
        cb = cst.tile([128, NCB], BF16, tag="cb", name="cb")
        nc.scalar.dma_start(cb[:], CB[:])
        cs8 = cst.tile([8, NCS], BF16, tag="cs8", name="cs8")
        nc.scalar.dma_start(cs8[:], CS8[:])
        xr_all = xp.tile([128, 8 * D], BF16, tag="xr", name="xr_all")
        nc.sync.dma_start(xr_all[:, 0:4 * D], xRb[:, 0:4 * D])
        nc.sync.dma_start(xr_all[:, 4 * D:], xRb[:, 4 * D:])
        # w1 in 2-ft chunks; descriptors are ISSUED LATER (closures invoked
        # at schedule points) so they don't block the softmax ACTs on the
        # scalar HWDGE queue
        w1_all = cst.tile([128, 4 * DFF], BF16, tag="w1", name="w1_all")
        w1v = w1_all[:].rearrange("p (t f j) -> p t f j", t=4, f=8)
        w1s = W1T.rearrange("(t p) (f j) -> p t f j", p=128, f=8)
        w1_load = [lambda ft=ft: nc.scalar.dma_start(
            w1v[:, :, ft, :], w1s[:, :, ft, :]) for ft in range(8)]
        # w2 in 4-ft chunks on the sync queue (needed last)
        w2_all = cst.tile([128, 16 * D], BF16, tag="w2", name="w2_all")
        w2v = w2_all[:].rearrange("p (f d) -> p f d", f=16)
        w2s = W2T.rearrange("(f p) d -> p f d", p=128)
        for ft in range(0, 16, 4):
            nc.sync.dma_start(w2v[:, ft:ft + 4, :], w2s[:, ft:ft + 4, :])

        eye_sb = cf[:, O_EYE:O_EYE + 128]
        wu_sb = cf[:, O_WU:O_WU + 16].bitcast(F16)        # [128, 32] f16
        cu8 = cr8[:, O_PEU:O_PEU + 4 * D].bitcast(F32)    # [8, 1024] f32
        eye8h = cr8[0:8, O_EY8:O_EY8 + 8]                 # [8, 8] f16
        Mstcb = cb[:, O_MST:O_MST + D]
        EYBcb = cb[:, O_EYB:O_EYB + 128]
        selTTb = cb[:, O_SELB:O_SELB + 8]                 # [128, 8] bf16
        epsT = cst.tile([128, 1], F32, tag="eps", name="epsT")
        nc.vector.memset(epsT[:], EPS)

        if not trivial:
            G1cb = cst.tile([128, D], BF16, tag="g1t", name="g1t")
            G2cb = cst.tile([128, D], BF16, tag="g2t", name="g2t")
            BE2cb = cst.tile([128, D], BF16, tag="be2t", name="be2t")
            B1Tcb = cst.tile([128, D], BF16, tag="b1tt", name="b1tt")
            for bt, off in ((G1cb, O_G1R), (G2cb, O_G2R),
                            (BE2cb, O_BE2R), (B1Tcb, O_B1TR)):
                nc.gpsimd.partition_broadcast(bt[:], cs8[0:1, off:off + D])

        # ---- ACT table preload (exp set) + PE warm-up during DMA wait.
        # N=512 warm matmuls: only a sustained wide stream flips HAM to
        # 8/8 (N=64 junk measurably never does); later warm_fill() calls
        # bridge dependency stalls in the attention phase so the clock
        # never re-throttles before the FFN stream takes over.
        junk = cst.tile([128, 1], F32, tag="junk", name="junk")
        nc.scalar.activation(junk[:], epsT[:], AF.Exp)
        warm_sb = cst.tile([128, D], BF16, tag="wrm", name="warm_sb")
        nc.vector.memset(warm_sb[:], 0.0)

        def warm_fill(n, cols=256):
            wt = ps.tile([128, D], F32, tag="wrm", bufs=1)
            for _ in range(n):
                nc.tensor.matmul(wt[:, 0:cols], warm_sb[:, 0:128],
                                 warm_sb[:, 0:cols], start=True, stop=True)

        warm_fill(10, cols=512)

        nrm1 = [qp.tile([128, D], BF16, tag=f"n1{m}", name=f"nrm1_{m}")
                for m in range(8)]
        if not trivial:
            o1_sb = [qp.tile([128, D], BF16, tag=f"o1{m}", name=f"o1sb{m}")
                     for m in range(8)]
        else:
            o1_sb = nrm1
        # o1T_all[p, m*512 + t*128 + y] = nrm1[m][y, t*128 + p]
        o1T_all = qp.tile([128, 8 * D], BF16, tag="oT", name="o1T_all")

        uT_sb = [qp.tile([8, D], F32, tag=f"uT{c}", name=f"uTsb{c}")
                 for c in range(2)]

        def uT_mm(c):
            """uT_ps[c][i, st*128+y] = sum_K x[row, K] * Wu[K, i] for the
            four tiles st of chunk c (cols 256*j from xq[2c+j])."""
            ups = ps.tile([8, D], F32, tag="mmA", bufs=2)
            # j=1 range: start=False on untouched PSUM (has_written unset
            # -> overwrite), so j=0's accumulating values aren't clobbered
            for j in range(2):
                for t in range(4):
                    nc.tensor.matmul(
                        ups[:, j * 256:(j + 1) * 256],
                        wu_sb[:, t * 8:(t + 1) * 8],
                        xq[2 * c + j][:, t * 256:(t + 1) * 256],
                        start=(j == 0 and t == 0), stop=(t == 3),
                        skip_group_check=True)
            # exact pe segment sums added on DVE (f16 would cost ~0.02 abs)
            nc.vector.tensor_add(uT_sb[c][:], ups[:],
                                 cu8[:, c * D:(c + 1) * D])

        def attn_softmax(c):
            mx = wk.tile([8, 1], F32, tag="mx")
            nc.vector.tensor_reduce(mx[:], uT_sb[c][:], axis=AX.X, op=ALU.max)
            nmx = wk.tile([8, 1], F32, tag="nmx")
            nc.vector.tensor_scalar_mul(nmx[:], mx[:], -1.0)
            ex = wk.tile([8, D], F32, tag=f"ex{c}", bufs=1)
            ssum = wk.tile([8, 1], F32, tag="esum")
            nc.scalar.activation(ex[:], uT_sb[c][:], AF.Exp, bias=nmx[:, :],
                                 accum_out=ssum[:])
            rcp = wk.tile([8, 1], F32, tag=f"ercp{c}", bufs=1)
            nc.vector.reciprocal(rcp[:], ssum[:])
            return ex, rcp          # ex is UNNORMALIZED; rcp folded into las

        def attn_v(c, ex, rcp, gaps=()):
            """las[i,:] = (xa_i @ Wq.T + peA_i) segments combined with M2,
            where xa_i = sum_s ex_i[s] x[s,:], all normalized by rcp[i].
            gaps: callables run before each dependency-gated PE group
            (warm fills for c0, FFN h1 quads for c1)."""
            gi = iter(gaps)
            gap = lambda: next(gi, lambda: None)()
            gap()
            # aT: ex [8,512] -> one [128,32] psum via 4 transposes + 1 copy
            atp = ps.tile([128, 32], F32, tag="vc", bufs=3)
            for st in range(4):
                nc.tensor.transpose(atp[:, st * 8:(st + 1) * 8],
                                    ex[:, st * 128:(st + 1) * 128],
                                    eye_sb[:8, :8])
            aT32 = wk.tile([128, 32], BF16, tag=f"aT{c}", bufs=1)
            nc.vector.tensor_copy(aT32[:], atp[:])
            aTss = [aT32[:, st * 8:(st + 1) * 8] for st in range(4)]
            gap()
            # xa[i, K] = sum_s a_i[s] x[s, K]   (contract s on the PE)
            xa = ps.tile([8, D], F32, tag="vc", bufs=3)
            for st in range(4):
                nc.tensor.matmul(
                    xa[:], aTss[st],
                    xr_all[:, (c * 4 + st) * D:(c * 4 + st + 1) * D],
                    start=(st == 0), stop=(st == 3))
            xas = wk.tile([8, D], F16, tag=f"xas{c}", bufs=1)
            nc.scalar.copy(xas[:], xa[:])
            # group sums gT[j, i] = sum_{s in group j of tile st} a_i[s]
            gt_ps = ps.tile([8, 32], F32, tag="vc", bufs=3)
            for st in range(4):
                nc.tensor.matmul(gt_ps[:, st * 8:(st + 1) * 8],
                                 selTTb, aTss[st], start=True, stop=True)
            gts = wk.tile([8, 32], F16, tag=f"gts{c}", bufs=1)
            nc.vector.tensor_copy(gts[:], gt_ps[:])
            gap()
            # xaT: 4 transposes [8,128] -> [128,8] f16 into one psum tile
            xat_ps = ps.tile([128, 32], F16, tag="vc", bufs=3)
            for t in range(4):
                nc.tensor.transpose(xat_ps[:, t * 8:(t + 1) * 8],
                                    xas[:, t * 128:(t + 1) * 128], eye8h)
            xat = wk.tile([128, 32], F16, tag=f"xat{c}", bufs=1)
            nc.vector.tensor_copy(xat[:], xat_ps[:])
            gap()
            # v[i, :] = sum_t xaT_t.T @ wq_t  +  sum_st gT_st.T @ peT_st
            v_ps = ps.tile([8, D], F32, tag="vc", bufs=3)
            for t in range(4):
                nc.tensor.matmul(v_ps[:], xat[:, t * 8:(t + 1) * 8],
                                 wq_all[:, t * D:(t + 1) * D],
                                 start=(t == 0), stop=False)
            for st in range(4):
                m = c * 4 + st
                nc.tensor.matmul(
                    v_ps[:], gts[:, st * 8:(st + 1) * 8],
                    cr8[:, O_PET + m * D:O_PET + (m + 1) * D],
                    start=False, stop=(st == 3))
            vs = wk.tile([8, D], F32, tag=f"vs{c}", bufs=1)
            nc.scalar.copy(vs[:], v_ps[:])
            gap()
            # vm[p, 2jt + p//64] = v[2jt + p//64, jt*128+p]  (head segments)
            vtp = ps.tile([128, 32], F32, tag="vc", bufs=3)
            for jt in range(4):
                nc.tensor.transpose(vtp[:, jt * 8:(jt + 1) * 8],
                                    vs[:, jt * 128:(jt + 1) * 128],
                                    eye_sb[:8, :8])
            vm = wk.tile([128, 8], BF16, tag=f"vm{c}", bufs=1)
            nc.vector.memset(vm[:], 0.0)
            for jt in range(4):
                nc.vector.tensor_copy(
                    vm[0:64, 2 * jt:2 * jt + 1],
                    vtp[0:64, jt * 8 + 2 * jt:jt * 8 + 2 * jt + 1])
                nc.vector.tensor_copy(
                    vm[64:128, 2 * jt + 1:2 * jt + 2],
                    vtp[64:128, jt * 8 + 2 * jt + 1:jt * 8 + 2 * jt + 2])
            gap()
            lap = ps.tile([8, D], F32, tag="vc", bufs=3)
            nc.tensor.matmul(lap[:], vm[:], Mstcb, start=True, stop=True)
            las = wk.tile([8, D], BF16, tag=f"las{c}", bufs=1)
            nc.vector.tensor_scalar_mul(las[:], lap[:], rcp[:])  # 1/Z here
            return las

        def ln_rsd(z):
            """bn stats -> (mu, rsd = 1/sqrt(var+eps)) from tile/PSUM z."""
            st6 = wk.tile([128, 6], F32, tag="ls")
            nc.vector.bn_stats(st6[:], z[:])
            mv = wk.tile([128, 2], F32, tag="lm")
            nc.vector.bn_aggr(mv[:], st6[:])
            sd = wk.tile([128, 1], F32, tag="lsd")
            nc.scalar.activation(sd[:], mv[:, 1:2], AF.Sqrt, bias=epsT[:, :])
            rsd = wk.tile([128, 1], F32, tag="lr")
            nc.vector.reciprocal(rsd[:], sd[:])
            return mv, rsd

        bcp_map = {}

        def attn_resid_bcp(c, las, jt):
            m = c * 4 + jt
            bcp = ps.tile([128, D], F32, tag="mmB", bufs=2)
            bcp_map[m] = bcp
            nc.tensor.matmul(bcp[:],
                             cs8[0:8, O_E8 + jt * 128:O_E8 + (jt + 1) * 128],
                             las[:], start=True, stop=False)
            # z1 += x residual on the PE (keeps the DVE chain short)
            nc.tensor.matmul(bcp[:], EYBcb, xr_all[:, m * D:(m + 1) * D],
                             start=False, stop=True)

        def attn_resid_fin(c, jt):
            m = c * 4 + jt
            bcp = bcp_map[m]
            mv, rsd = ln_rsd(bcp)
            nc.vector.tensor_scalar(nrm1[m][:], bcp[:], mv[:, 0:1], rsd[:],
                                    op0=ALU.subtract, op1=ALU.mult)
            if not trivial:
                nc.gpsimd.tensor_mul(o1_sb[m][:], nrm1[m][:], G1cb[:])
                nc.gpsimd.tensor_add(o1_sb[m][:], o1_sb[m][:], B1Tcb[:])

        def attn_resid_tp(c, jt):
            # o1T via 4 PE transposes into one PSUM bank + a single copy
            m = c * 4 + jt
            tps = ps.tile([128, D], BF16, tag="vc", bufs=3)
            for t in range(4):
                nc.tensor.transpose(tps[:, t * 128:(t + 1) * 128],
                                    nrm1[m][:, t * 128:(t + 1) * 128], EYBcb)
            dst = o1T_all[:, m * D:(m + 1) * D]
            if m % 2 == 0:
                nc.scalar.copy(dst, tps[:])
            else:
                nc.vector.tensor_copy(dst, tps[:])

        h1map = {}

        # strided rhs view: oTr[p, m, t, y] = o1T_all[p, m*512 + t*128 + y]
        oTr = o1T_all[:].rearrange("p (m t y) -> p m t y", m=8, t=4)

        def ffn_h1(h, ft, split=False):
            p1 = ps.tile([128, D], F32, tag="mmA", bufs=2)
            if split:
                # two N=256 halves: the first needs only m-tiles (h*4, h*4+1)
                for half in range(2):
                    for dt in range(4):
                        nc.tensor.matmul(
                            p1[:, half * 256:(half + 1) * 256],
                            w1_all[:, dt * DFF + ft * 128:
                                   dt * DFF + (ft + 1) * 128],
                            oTr[:, h * 4 + 2 * half:h * 4 + 2 * half + 2,
                                dt, :],
                            start=(half == 0 and dt == 0), stop=(dt == 3),
                            skip_group_check=True)
            else:
                for dt in range(4):
                    nc.tensor.matmul(
                        p1[:],
                        w1_all[:, dt * DFF + ft * 128:dt * DFF + (ft + 1) * 128],
                        oTr[:, h * 4:(h + 1) * 4, dt, :],
                        start=(dt == 0), stop=(dt == 3))
            h1t = hp.tile([128, D], BF16, tag=f"h1_{ft}", bufs=2,
                          name=f"h1_{h}_{ft}")
            nc.scalar.activation(h1t[:], p1[:], AF.Relu,
                                 bias=cf[:, O_B1P + ft:O_B1P + ft + 1])
            h1map[(h, ft)] = h1t

        def ffn_rm(m):
            h, rm = divmod(m, 4)
            tail = m >= 6
            p2 = ps.tile([128, D], F32, tag="mmB", bufs=2)
            for ft in range(16):
                nc.tensor.matmul(
                    p2[:], h1map[(h, ft)][:, rm * 128:(rm + 1) * 128],
                    w2_all[:, ft * D:(ft + 1) * D],
                    start=(ft == 0), stop=(ft == 15 and not tail))
            if tail:
                # z2 += o1 on the (tail-idle) PE; LN2 reads PSUM directly
                nc.tensor.matmul(p2[:], EYBcb, o1_sb[m][:],
                                 start=False, stop=True)
                z2 = p2
            else:
                z2t = wk.tile([128, D], BF16, tag="z2")
                nc.vector.tensor_add(z2t[:], p2[:], o1_sb[m][:])
                z2 = z2t
            mv, rsd = ln_rsd(z2)
            if trivial:
                yt = wk.tile([128, D], BF16, tag="yt")
                nc.vector.tensor_scalar(yt[:], z2[:], mv[:, 0:1], rsd[:],
                                        op0=ALU.subtract, op1=ALU.mult)
            else:
                nrm2 = wk.tile([128, D], BF16, tag="n2")
                nc.vector.tensor_scalar(nrm2[:], z2[:], mv[:, 0:1], rsd[:],
                                        op0=ALU.subtract, op1=ALU.mult)
                tg = wk.tile([128, D], BF16, tag="tg")
                yt = wk.tile([128, D], BF16, tag="yt")
                if tail:
                    nc.vector.tensor_mul(tg[:], nrm2[:], G2cb[:])
                    nc.vector.tensor_add(yt[:], tg[:], BE2cb[:])
                else:
                    nc.gpsimd.tensor_mul(tg[:], nrm2[:], G2cb[:])
                    nc.gpsimd.tensor_add(yt[:], tg[:], BE2cb[:])
            nc.sync.dma_start(out[m * 128:(m + 1) * 128, :], yt[:])

        # ---------------- schedule ----------------
        fill = lambda n: (lambda: warm_fill(n))
        uT_mm(0)
        warm_fill(6)
        uT_mm(1)
        warm_fill(10)
        ex0, rcp0 = attn_softmax(0)
        ex1, rcp1 = attn_softmax(1)
        # preload the sqrt-family table set: reading ex1 (written by the
        # second Exp on this same queue) forces this AFTER both Exps
        junk8 = cst.tile([8, 1], F32, tag="junk8", name="junk8")
        nc.scalar.activation(junk8[:], ex1[:, 0:1], AF.Sqrt)
        w1_load[0]()
        w1_load[1]()
        las0 = attn_v(0, ex0, rcp0,
                      gaps=[fill(4), fill(4), fill(4), fill(4), fill(4),
                            fill(3)])
        w1_load[2]()
        w1_load[3]()
        warm_fill(3)
        for jt in range(4):
            attn_resid_bcp(0, las0, jt)
        for jt in range(4):
            attn_resid_fin(0, jt)
        w1_load[4]()
        w1_load[5]()
        warm_fill(8)
        attn_resid_tp(0, 0)
        warm_fill(3)
        attn_resid_tp(0, 1)
        warm_fill(3)
        attn_resid_tp(0, 2)
        attn_resid_tp(0, 3)
        w1_load[6]()
        w1_load[7]()

        def h1q(h, lo, hi, split):
            def go():
                for ft in range(lo, hi):
                    ffn_h1(h, ft, split=split)
            return go

        las1 = attn_v(1, ex1, rcp1,
                      gaps=[h1q(0, 0, 2, True), h1q(0, 2, 4, True),
                            h1q(0, 4, 6, True), h1q(0, 6, 8, True),
                            h1q(0, 8, 10, True), h1q(0, 10, 12, True)])
        for jt in range(4):
            attn_resid_bcp(1, las1, jt)
            if jt == 0:
                h1q(0, 12, 14, True)()
        for jt in range(4):
            attn_resid_fin(1, jt)
        h1q(0, 14, 16, True)()
        attn_resid_tp(1, 0)
        attn_resid_tp(1, 1)
        ffn_rm(0)
        attn_resid_tp(1, 2)
        attn_resid_tp(1, 3)
        h1q(1, 0, 4, False)()
        ffn_rm(1)
        h1q(1, 4, 8, False)()
        ffn_rm(2)
        h1q(1, 8, 12, False)()
        ffn_rm(3)
        h1q(1, 12, 16, False)()
        for m in range(4, 8):
            ffn_rm(m)

    nc.compile()
    return nc


def _pe_table():
    pos = np.arange(S, dtype=np.float32)[:, None]
    div = np.exp(np.arange(0, D, 2, dtype=np.float32) * (-math.log(10000.0) / D))
    ang = pos * div
    pe = np.zeros((S, D), np.float32)
    pe[:, 0::2] = np.sin(ang)
    pe[:, 1::2] = np.cos(ang)
    return pe


def _is_trivial(W1, b1, b2, g1, be1, g2, be2):
    f32 = lambda a: np.asarray(a, dtype=np.float32)
    return (np.all(f32(g1) == 1) and np.all(f32(be1) == 0)
            and np.all(f32(b2) == 0) and np.all(f32(g2) == 1)
            and np.all(f32(be2) == 0))


def make_in_maps(x, Wq, Wfc, W1, b1, W2, b2, g1, be1, g2, be2):
    f32 = lambda a: np.ascontiguousarray(a, dtype=np.float32)
    bfc = lambda a: np.ascontiguousarray(np.asarray(f32(a), dtype="bfloat16"))
    xf = f32(x).reshape(S * H * W, D)
    pe = _pe_table()
    M2 = f32(Wfc).reshape(D, NH, DEP).sum(axis=1).T          # (64, 512)
    Mstk = np.concatenate([M2, M2], axis=0)                  # (128, 512)

    WqTf = f32(Wq.T)
    Wu = WqTf.reshape(D, NH, DEP).sum(axis=2)                # (K, i)
    WUc = np.ascontiguousarray(
        Wu.reshape(4, 128, NH).transpose(1, 0, 2).reshape(128, 32),
        dtype=np.float16)

    CF = np.zeros((128, NCF), np.float32)
    CF[:, O_EYE:O_EYE + 128] = np.eye(128, dtype=np.float32)
    b1p = f32(b1) + f32(W1) @ f32(be1)                       # be1 folded
    CF[:, O_B1P:O_B1P + 16] = b1p.reshape(16, 128).T
    CF[:, O_WU:O_WU + 16] = WUc.view(np.float32)

    selTT = np.zeros((128, 8), np.float32)                   # [s, j]
    for p in range(128):
        selTT[p, p // 16] = 1.0

    CB = np.zeros((128, NCB), np.float32)
    CB[:, O_EYB:O_EYB + 128] = np.eye(128, dtype=np.float32)
    CB[:, O_MST:O_MST + D] = Mstk
    CB[:, O_SELB:O_SELB + 8] = selTT

    CS = np.zeros((8, NCS), np.float32)
    for jt in range(4):
        for p in range(128):
            CS[2 * jt + p // 64, O_E8 + jt * 128 + p] = 1.0
    CS[0, O_G1R:O_G1R + D] = f32(g1)
    CS[0, O_G2R:O_G2R + D] = f32(g2)
    CS[0, O_BE2R:O_BE2R + D] = f32(be2)
    CS[0, O_B1TR:O_B1TR + D] = f32(b2) + f32(be1)

    shared = dict(
        WqT=np.ascontiguousarray(WqTf, dtype=np.float16),
        W1T=bfc(f32(W1) * f32(g1)[None, :]).T.copy(),        # g1 folded
        W2T=bfc(f32(W2).T),
        CF=CF, CB=bfc(CB), CS8=bfc(CS),
    )
    selT = np.zeros((8, 128), np.float32)
    for rr in range(128):
        selT[rr // 16, rr] = 1.0

    peu_all = pe.reshape(S, NH, DEP).sum(axis=2)             # (S, 8) f32

    maps = []
    for k in range(NCORES):
        sl = xf[k * R:(k + 1) * R]
        m = dict(shared)
        slT = np.asarray(sl.T, dtype=np.float16)
        # xq layout: row-block i = m-pair (2i, 2i+1); columns (dt, mi, c)
        arr = slT.reshape(4, 128, 4, 2, 128)        # (t, p, i, mi, c)
        arr = arr.transpose(2, 1, 0, 3, 4)          # (i, p, t, mi, c)
        m["xT"] = np.ascontiguousarray(arr.reshape(512, 1024))
        # xRb[p, t*512+d] = x[t*128+p, d]
        m["xRb"] = bfc(sl.reshape(8, 128, D).transpose(1, 0, 2)
                       .reshape(128, 8 * D))
        cr8 = np.zeros((8, NCR8), np.float16)
        cr8[:, O_SEL:O_SEL + 128] = selT
        pe_loc = pe[k * 64:(k + 1) * 64]            # (64, 512)
        cr8[:, O_PET:O_PET + 8 * D] = (pe_loc.reshape(8, 8, D)
                                       .transpose(1, 0, 2).reshape(8, 8 * D))
        cr8[:, O_EY8:O_EY8 + 8] = np.eye(8, dtype=np.float16)
        # pe segment sums, exact f32, laid out [i, c*512 + st*128 + y]
        peu_loc = peu_all[k * 64:(k + 1) * 64]      # (64, 8)
        cu = np.zeros((8, 2 * D), np.float32)
        for mm in range(8):
            c, st = divmod(mm, 4)
            blk = peu_loc[mm * 8:(mm + 1) * 8]      # (j, i)
            cu[:, c * D + st * 128:c * D + (st + 1) * 128] = blk.T @ selT
        cr8[:, O_PEU:O_PEU + 4 * D] = cu.view(np.float16)
        m["CR8"] = np.ascontiguousarray(cr8)
        maps.append(m)
    return maps


def kernel(x, Wq, Wfc, W1, b1, W2, b2, g1, be1, g2, be2, _results_hook=None,
           _trace=False, _tmpdir=None):
    trivial = _is_trivial(W1, b1, b2, g1, be1, g2, be2)
    key = ("nc", trivial)
    if key not in _cached:
        _cached[key] = build_nc(trivial)
    nc = _cached[key]
    in_maps = make_in_maps(x, Wq, Wfc, W1, b1, W2, b2, g1, be1, g2, be2)
    res = run_bass_kernel_spmd(nc, in_maps, list(range(NCORES)),
                               trace=_trace, tmpdir=_tmpdir)
    if _results_hook is not None:
        _results_hook(res)
    y = np.concatenate([np.asarray(res.results[k]["out"], dtype=np.float32)
                        for k in range(NCORES)], axis=0)
    return y.reshape(S, H, W, D)


# revision 21
# speedup vs baseline: 1.2005x; 1.0051x over previous
"""Trainium2 Bass kernel for nn_EncoderLayer_73315091743398.

The reference module's attention einsums ('hwink,hwijm->hwinm') sum their k/j
indices independently, so the whole attention block collapses to, per
(h,w)-chunk c and head i, over the flat q matrix qf = x@Wq.T + pe viewed as
(8192, 512) in raw (s,h,w) row order:

    u[s]  = sum_d qf[c*512+s, 64i+d]          (segment row sums)
    a     = softmax_s(u)
    v[d]  = sum_s a[s] * qf[c*512+s, 64i+d]
    row   = tile8(v) @ Wfc.T = v @ M,  M[d,:] = sum_b Wfc[:, 64b+d].T

and attn_out viewed (S,H,W,D) has row A[s'] = row_{c=s'//32, i=(s'%32)//4},
independent of (h,w).  Core k owns raw rows [k*1024,(k+1)*1024): these are
exactly attention chunks {2k, 2k+1} AND the residual/FFN rows for
s' in [64k, 64k+64), so the 8 cores run fully independent SPMD programs.

v3 structure (vs the 120.7us baseline):
  - uT[i,s] is computed DIRECTLY as x @ Wu (Wu = segment-summed Wq.T
    columns) into an [8,512] PSUM per chunk + an exact f32 pe-segment-sum
    added on the DVE; softmax for both chunks runs early and entirely off
    the FFN critical path.
  - The value pass never materializes q: by associativity,
      v_i = (sum_s a_i[s] x[s,:]) @ Wq.T + (sum_s a_i[s] pe[s,:])
    so per chunk: xa = aT.T @ x (4 matmuls), v = xaT @ WqT (4) plus the
    pe term via group-sum selectors (8 tiny).  This deletes the 40 q
    matmuls and the 8 scalar-engine PSUM->SBUF q copies of the q path.
  - 1/sum(exp) is folded into the las row scale (tensor_scalar_mul),
    removing the ex normalization from the a-tensor critical path.
  - Exp and Sqrt ACT table sets would thrash (different sets, ~2.7us per
    switch): both softmax Exps are issued back-to-back, then a junk Sqrt
    that READS the second exp's output (ordering-proof) preloads the
    sqrt set once for all 16 LN chains.
  - o1T for the FFN is built with 4 PE transposes per tile into one PSUM
    bank + a single copy, replacing DMA transposes (1.2-1.5us descriptor
    stalls on the HWDGE queues).
  - Input DMA descriptor generation (~0.6-0.8us each on an HWDGE queue!)
    is split across BOTH queues (sync + scalar), small consts are packed
    into two tensors (bitcast views), and w1/w2 arrive in per-ft chunks
    so the FFN streams against DMA arrival.
  - Trivial LN affine params / biases (true for this problem's inputs)
    drop the GpSimd scale/bias ops; output is bf16, upcast on host.
"""

import math
import os
import sys
from contextlib import ExitStack

import numpy as np
import ml_dtypes  # noqa: F401  (registers bfloat16)

for _p in ("/opt/trn_rl_repo", "/root/.axon_site/_ro/trn_rl_repo"):
    if os.path.isdir(_p) and _p not in sys.path:
        sys.path.append(_p)

import concourse.bass as bass
import concourse.bacc as bacc
import concourse.mybir as mybir
import concourse.tile as tile
from concourse.bass_utils import run_bass_kernel_spmd

F32 = mybir.dt.float32
F16 = mybir.dt.float16
BF16 = mybir.dt.bfloat16
AF = mybir.ActivationFunctionType
ALU = mybir.AluOpType
AX = mybir.AxisListType

S, H, W, D = 512, 4, 4, 512
NH, DEP, DFF = 8, 64, 2048
NCORES = 8
R = 1024          # rows per core of the flat (8192, 512) view
EPS = 1e-5

# CF (f32): eye128 [128], b1p [16], wu-bitcast [16] -> 160
O_EYE, O_B1P, O_WU = 0, 128, 144
NCF = 160
# CB (bf16): eye128 + Mst + selTTb [8]
O_EYB, O_MST, O_SELB = 0, 128, 640
NCB = 648
# CS8 (bf16, 8 partitions): E8 selector rows + rows of g1/g2/be2/(b2+be1)
O_E8, O_G1R, O_G2R, O_BE2R, O_B1TR = 0, 512, 1024, 1536, 2048
NCS = 2560
# CR8 (fp16, 8 partitions): selT [8,128], peT [8,8*512], eye8 [8,8],
# peuT-bitcast [8, 2*512 f32]
O_SEL, O_PET, O_EY8, O_PEU = 0, 128, 128 + 8 * D, 128 + 8 * D + 8
NCR8 = O_PEU + 4 * D          # 2*512 f32 as 4*512 f16 halves

_cached = {}


def build_nc(trivial):
    """Single-core SPMD Bass/Tile program (same program on all 8 cores).

    trivial=True: g1==1, be1==0, b2==0, g2==1, be2==0 (the actual inputs);
    drops the residual scale and the output scale/bias ops.
    """
    nc = bacc.Bacc("TRN2", debug=False, target_bir_lowering=False)

    xT = nc.dram_tensor("xT", [D, R], F16, kind="ExternalInput")
    xRb = nc.dram_tensor("xRb", [128, 8 * D], BF16, kind="ExternalInput")
    WqT = nc.dram_tensor("WqT", [D, D], F16, kind="ExternalInput")
    W1T = nc.dram_tensor("W1T", [D, DFF], BF16, kind="ExternalInput")
    W2T = nc.dram_tensor("W2T", [DFF, D], BF16, kind="ExternalInput")
    CF = nc.dram_tensor("CF", [128, NCF], F32, kind="ExternalInput")
    CB = nc.dram_tensor("CB", [128, NCB], BF16, kind="ExternalInput")
    CS8 = nc.dram_tensor("CS8", [8, NCS], BF16, kind="ExternalInput")
    CR8 = nc.dram_tensor("CR8", [8, NCR8], F16, kind="ExternalInput")
    out = nc.dram_tensor("out", [R, D], BF16, kind="ExternalOutput")

    with ExitStack() as ctx:
        tc = ctx.enter_context(tile.TileContext(nc))
        cst = ctx.enter_context(tc.tile_pool(name="cst", bufs=1))
        xp = ctx.enter_context(tc.tile_pool(name="xp", bufs=1))
        qp = ctx.enter_context(tc.tile_pool(name="qp", bufs=1))
        hp = ctx.enter_context(tc.tile_pool(name="hp", bufs=1))
        wk = ctx.enter_context(tc.tile_pool(name="wk", bufs=2))
        ps = ctx.enter_context(tc.tile_pool(name="ps", bufs=1, space="PSUM"))

        # ---- loads, split across BOTH HWDGE queues (descriptor generation
        #      is ~0.6-0.8us each and would serialize on one queue).
        # sync queue: cr8(+pe sums), xq0..3, xr halves, w2 chunks
        # scalar queue: cf(+wu), wq, cb, cs8, w1 chunks
        cr8 = cst.tile([8, NCR8], F16, tag="cr8", name="cr8")
        nc.sync.dma_start(cr8[:], CR8[:])
        xq = [xp.tile([128, R], F16, tag=f"dT{i}", name=f"xq{i}")
              for i in range(4)]
        nc.sync.dma_start(xq[0][:], xT[0:128, :])
        nc.sync.dma_start(xq[1][:], xT[128:256, :])
        cf = cst.tile([128, NCF], F32, tag="cf", name="cf")
        nc.sync.dma_start(cf[:], CF[:])
        nc.sync.dma_start(xq[2][:], xT[256:384, :])
        nc.sync.dma_start(xq[3][:], xT[384:512, :])
        xr_all = xp.tile([128, 8 * D], BF16, tag="xr", name="xr_all")
        nc.sync.dma_start(xr_all[:, 0:4 * D], xRb[:, 0:4 * D])
        wq_all = cst.tile([128, 4 * D], F16, tag="wq", name="wq_all")
        nc.sync.dma_start(
            wq_all[:].rearrange("p (t j) -> p t j", t=4),
            WqT.rearrange("(t p) j -> p t j", p=128))
        cb = cst.tile([128, NCB], BF16, tag="cb", name="cb")
        nc.sync.dma_start(cb[:], CB[:])
        cs8 = cst.tile([8, NCS], BF16, tag="cs8", name="cs8")
        nc.sync.dma_start(cs8[:], CS8[:])
        nc.sync.dma_start(xr_all[:, 4 * D:], xRb[:, 4 * D:])
        # w1 in 2-ft chunks, w2 in 4-ft chunks: the FFN gates per chunk
        w1_all = cst.tile([128, 4 * DFF], BF16, tag="w1", name="w1_all")
        w1v = w1_all[:].rearrange("p (t f j) -> p t f j", t=4, f=8)
        w1s = W1T.rearrange("(t p) (f j) -> p t f j", p=128, f=8)
        for ft in range(8):
            nc.sync.dma_start(w1v[:, :, ft, :], w1s[:, :, ft, :])
        w2_all = cst.tile([128, 16 * D], BF16, tag="w2", name="w2_all")
        w2v = w2_all[:].rearrange("p (f d) -> p f d", f=16)
        w2s = W2T.rearrange("(f p) d -> p f d", p=128)
        for ft in range(0, 16, 4):
            nc.sync.dma_start(w2v[:, ft:ft + 4, :], w2s[:, ft:ft + 4, :])

        eye_sb = cf[:, O_EYE:O_EYE + 128]
        wu_sb = cf[:, O_WU:O_WU + 16].bitcast(F16)        # [128, 32] f16
        cu8 = cr8[:, O_PEU:O_PEU + 4 * D].bitcast(F32)    # [8, 1024] f32
        eye8h = cr8[0:8, O_EY8:O_EY8 + 8]                 # [8, 8] f16
        Mstcb = cb[:, O_MST:O_MST + D]
        EYBcb = cb[:, O_EYB:O_EYB + 128]
        selTTb = cb[:, O_SELB:O_SELB + 8]                 # [128, 8] bf16
        epsT = cst.tile([128, 1], F32, tag="eps", name="epsT")
        nc.vector.memset(epsT[:], EPS)

        if not trivial:
            G1cb = cst.tile([128, D], BF16, tag="g1t", name="g1t")
            G2cb = cst.tile([128, D], BF16, tag="g2t", name="g2t")
            BE2cb = cst.tile([128, D], BF16, tag="be2t", name="be2t")
            B1Tcb = cst.tile([128, D], BF16, tag="b1tt", name="b1tt")
            for bt, off in ((G1cb, O_G1R), (G2cb, O_G2R),
                            (BE2cb, O_BE2R), (B1Tcb, O_B1TR)):
                nc.gpsimd.partition_broadcast(bt[:], cs8[0:1, off:off + D])

        # ---- ACT table preload (exp set) + PE warm-up during DMA wait.
        # N=512 warm matmuls: only a sustained wide stream flips HAM to
        # 8/8 (N=64 junk measurably never does); later warm_fill() calls
        # bridge dependency stalls in the attention phase so the clock
        # never re-throttles before the FFN stream takes over.
        junk = cst.tile([128, 1], F32, tag="junk", name="junk")
        nc.scalar.activation(junk[:], epsT[:], AF.Exp)
        warm_sb = cst.tile([128, D], BF16, tag="wrm", name="warm_sb")
        nc.vector.memset(warm_sb[:], 0.0)

        def warm_fill(n, cols=256):
            wt = ps.tile([128, D], F32, tag="wrm", bufs=1)
            for _ in range(n):
                nc.tensor.matmul(wt[:, 0:cols], warm_sb[:, 0:128],
                                 warm_sb[:, 0:cols], start=True, stop=True)

        warm_fill(10, cols=512)

        nrm1 = [qp.tile([128, D], BF16, tag=f"n1{m}", name=f"nrm1_{m}")
                for m in range(8)]
        if not trivial:
            o1_sb = [qp.tile([128, D], BF16, tag=f"o1{m}", name=f"o1sb{m}")
                     for m in range(8)]
        else:
            o1_sb = nrm1
        # o1T_all[p, m*512 + t*128 + y] = nrm1[m][y, t*128 + p]
        o1T_all = qp.tile([128, 8 * D], BF16, tag="oT", name="o1T_all")

        uT_sb = [qp.tile([8, D], F32, tag=f"uT{c}", name=f"uTsb{c}")
                 for c in range(2)]

        def uT_mm(c):
            """uT_ps[c][i, st*128+y] = sum_K x[row, K] * Wu[K, i] for the
            four tiles st of chunk c (cols 256*j from xq[2c+j])."""
            ups = ps.tile([8, D], F32, tag="mmA", bufs=2)
            # j=1 range: start=False on untouched PSUM (has_written unset
            # -> overwrite), so j=0's accumulating values aren't clobbered
            for j in range(2):
                for t in range(4):
                    nc.tensor.matmul(
                        ups[:, j * 256:(j + 1) * 256],
                        wu_sb[:, t * 8:(t + 1) * 8],
                        xq[2 * c + j][:, t * 256:(t + 1) * 256],
                        start=(j == 0 and t == 0), stop=(t == 3),
                        skip_group_check=True)
            # exact pe segment sums added on DVE (f16 would cost ~0.02 abs)
            nc.vector.tensor_add(uT_sb[c][:], ups[:],
                                 cu8[:, c * D:(c + 1) * D])

        def attn_softmax(c):
            mx = wk.tile([8, 1], F32, tag="mx")
            nc.vector.tensor_reduce(mx[:], uT_sb[c][:], axis=AX.X, op=ALU.max)
            nmx = wk.tile([8, 1], F32, tag="nmx")
            nc.vector.tensor_scalar_mul(nmx[:], mx[:], -1.0)
            ex = wk.tile([8, D], F32, tag=f"ex{c}", bufs=1)
            ssum = wk.tile([8, 1], F32, tag="esum")
            nc.scalar.activation(ex[:], uT_sb[c][:], AF.Exp, bias=nmx[:, :],
                                 accum_out=ssum[:])
            rcp = wk.tile([8, 1], F32, tag=f"ercp{c}", bufs=1)
            nc.vector.reciprocal(rcp[:], ssum[:])
            return ex, rcp          # ex is UNNORMALIZED; rcp folded into las

        def attn_v(c, ex, rcp, gaps=(), post_xas=None, post_vs=None):
            """las[i,:] = (xa_i @ Wq.T + peA_i) segments combined with M2,
            where xa_i = sum_s ex_i[s] x[s,:], all normalized by rcp[i].
            gaps: callables run before each dependency-gated PE group
            (warm fills for c0, FFN h1 quads for c1)."""
            gi = iter(gaps)
            gap = lambda: next(gi, lambda: None)()
            gap()
            # aT: ex [8,512] -> one [128,32] psum via 4 transposes + 1 copy
            atp = ps.tile([128, 32], F32, tag="vc", bufs=3)
            for st in range(4):
                nc.tensor.transpose(atp[:, st * 8:(st + 1) * 8],
                                    ex[:, st * 128:(st + 1) * 128],
                                    eye_sb[:8, :8])
            aT32 = wk.tile([128, 32], BF16, tag=f"aT{c}", bufs=1)
            nc.vector.tensor_copy(aT32[:], atp[:])
            aTss = [aT32[:, st * 8:(st + 1) * 8] for st in range(4)]
            gap()
            # xa[i, K] = sum_s a_i[s] x[s, K]   (contract s on the PE)
            xa = ps.tile([8, D], F32, tag="vc", bufs=3)
            for st in range(4):
                nc.tensor.matmul(
                    xa[:], aTss[st],
                    xr_all[:, (c * 4 + st) * D:(c * 4 + st + 1) * D],
                    start=(st == 0), stop=(st == 3))
            xas = wk.tile([8, D], F16, tag=f"xas{c}", bufs=1)
            nc.scalar.copy(xas[:], xa[:])
            if post_xas is not None:
                post_xas()
            # group sums gT[j, i] = sum_{s in group j of tile st} a_i[s]
            gt_ps = ps.tile([8, 32], F32, tag="vc", bufs=3)
            for st in range(4):
                nc.tensor.matmul(gt_ps[:, st * 8:(st + 1) * 8],
                                 selTTb, aTss[st], start=True, stop=True)
            gts = wk.tile([8, 32], F16, tag=f"gts{c}", bufs=1)
            nc.vector.tensor_copy(gts[:], gt_ps[:])
            gap()
            # xaT: 4 transposes [8,128] -> [128,8] f16 into one psum tile
            xat_ps = ps.tile([128, 32], F16, tag="vc", bufs=3)
            for t in range(4):
                nc.tensor.transpose(xat_ps[:, t * 8:(t + 1) * 8],
                                    xas[:, t * 128:(t + 1) * 128], eye8h)
            xat = wk.tile([128, 32], F16, tag=f"xat{c}", bufs=1)
            nc.vector.tensor_copy(xat[:], xat_ps[:])
            gap()
            # v[i, :] = sum_t xaT_t.T @ wq_t  +  sum_st gT_st.T @ peT_st
            v_ps = ps.tile([8, D], F32, tag="vc", bufs=3)
            for t in range(4):
                nc.tensor.matmul(v_ps[:], xat[:, t * 8:(t + 1) * 8],
                                 wq_all[:, t * D:(t + 1) * D],
                                 start=(t == 0), stop=False)
            for st in range(4):
                m = c * 4 + st
                nc.tensor.matmul(
                    v_ps[:], gts[:, st * 8:(st + 1) * 8],
                    cr8[:, O_PET + m * D:O_PET + (m + 1) * D],
                    start=False, stop=(st == 3))
            vs = wk.tile([8, D], F32, tag=f"vs{c}", bufs=1)
            nc.scalar.copy(vs[:], v_ps[:])
            if post_vs is not None:
                post_vs()
            gap()
            # vm[p, 2jt + p//64] = v[2jt + p//64, jt*128+p]  (head segments)
            vtp = ps.tile([128, 32], F32, tag="vc", bufs=3)
            for jt in range(4):
                nc.tensor.transpose(vtp[:, jt * 8:(jt + 1) * 8],
                                    vs[:, jt * 128:(jt + 1) * 128],
                                    eye_sb[:8, :8])
            vm = wk.tile([128, 8], BF16, tag=f"vm{c}", bufs=1)
            nc.vector.memset(vm[:], 0.0)
            for jt in range(4):
                nc.vector.tensor_copy(
                    vm[0:64, 2 * jt:2 * jt + 1],
                    vtp[0:64, jt * 8 + 2 * jt:jt * 8 + 2 * jt + 1])
                nc.vector.tensor_copy(
                    vm[64:128, 2 * jt + 1:2 * jt + 2],
                    vtp[64:128, jt * 8 + 2 * jt + 1:jt * 8 + 2 * jt + 2])
            gap()
            lap = ps.tile([8, D], F32, tag="vc", bufs=3)
            nc.tensor.matmul(lap[:], vm[:], Mstcb, start=True, stop=True)
            las = wk.tile([8, D], BF16, tag=f"las{c}", bufs=1)
            nc.vector.tensor_scalar_mul(las[:], lap[:], rcp[:])  # 1/Z here
            return las

        def ln_rsd(z):
            """bn stats -> (mu, rsd = 1/sqrt(var+eps)) from tile/PSUM z."""
            st6 = wk.tile([128, 6], F32, tag="ls")
            nc.vector.bn_stats(st6[:], z[:])
            mv = wk.tile([128, 2], F32, tag="lm")
            nc.vector.bn_aggr(mv[:], st6[:])
            sd = wk.tile([128, 1], F32, tag="lsd")
            nc.scalar.activation(sd[:], mv[:, 1:2], AF.Sqrt, bias=epsT[:, :])
            rsd = wk.tile([128, 1], F32, tag="lr")
            nc.vector.reciprocal(rsd[:], sd[:])
            return mv, rsd

        bcp_map = {}

        def attn_resid_bcp(c, las, jt):
            m = c * 4 + jt
            bcp = ps.tile([128, D], F32, tag="mmB", bufs=2)
            bcp_map[m] = bcp
            nc.tensor.matmul(bcp[:],
                             cs8[0:8, O_E8 + jt * 128:O_E8 + (jt + 1) * 128],
                             las[:], start=True, stop=False)
            # z1 += x residual on the PE (keeps the DVE chain short)
            nc.tensor.matmul(bcp[:], EYBcb, xr_all[:, m * D:(m + 1) * D],
                             start=False, stop=True)

        def attn_resid_fin(c, jt):
            m = c * 4 + jt
            bcp = bcp_map[m]
            mv, rsd = ln_rsd(bcp)
            nc.vector.tensor_scalar(nrm1[m][:], bcp[:], mv[:, 0:1], rsd[:],
                                    op0=ALU.subtract, op1=ALU.mult)
            if not trivial:
                nc.gpsimd.tensor_mul(o1_sb[m][:], nrm1[m][:], G1cb[:])
                nc.gpsimd.tensor_add(o1_sb[m][:], o1_sb[m][:], B1Tcb[:])

        def attn_resid_tp(c, jt):
            # o1T via 4 PE transposes into one PSUM bank + a single copy
            m = c * 4 + jt
            tps = ps.tile([128, D], BF16, tag="vc", bufs=3)
            for t in range(4):
                nc.tensor.transpose(tps[:, t * 128:(t + 1) * 128],
                                    nrm1[m][:, t * 128:(t + 1) * 128], EYBcb)
            dst = o1T_all[:, m * D:(m + 1) * D]
            if m % 2 == 0:
                nc.scalar.copy(dst, tps[:])
            else:
                nc.vector.tensor_copy(dst, tps[:])

        h1map = {}

        # strided rhs view: oTr[p, m, t, y] = o1T_all[p, m*512 + t*128 + y]
        oTr = o1T_all[:].rearrange("p (m t y) -> p m t y", m=8, t=4)

        def ffn_h1(h, ft, split=False):
            p1 = ps.tile([128, D], F32, tag="mmA", bufs=2)
            if split:
                # two N=256 halves: the first needs only m-tiles (h*4, h*4+1)
                for half in range(2):
                    for dt in range(4):
                        nc.tensor.matmul(
                            p1[:, half * 256:(half + 1) * 256],
                            w1_all[:, dt * DFF + ft * 128:
                                   dt * DFF + (ft + 1) * 128],
                            oTr[:, h * 4 + 2 * half:h * 4 + 2 * half + 2,
                                dt, :],
                            start=(half == 0 and dt == 0), stop=(dt == 3),
                            skip_group_check=True)
            else:
                for dt in range(4):
                    nc.tensor.matmul(
                        p1[:],
                        w1_all[:, dt * DFF + ft * 128:dt * DFF + (ft + 1) * 128],
                        oTr[:, h * 4:(h + 1) * 4, dt, :],
                        start=(dt == 0), stop=(dt == 3))
            h1t = hp.tile([128, D], BF16, tag=f"h1_{ft}", bufs=2,
                          name=f"h1_{h}_{ft}")
            nc.scalar.activation(h1t[:], p1[:], AF.Relu,
                                 bias=cf[:, O_B1P + ft:O_B1P + ft + 1])
            h1map[(h, ft)] = h1t

        def ffn_rm(m):
            h, rm = divmod(m, 4)
            tail = m >= 6
            p2 = ps.tile([128, D], F32, tag="mmB", bufs=2)
            for ft in range(16):
                nc.tensor.matmul(
                    p2[:], h1map[(h, ft)][:, rm * 128:(rm + 1) * 128],
                    w2_all[:, ft * D:(ft + 1) * D],
                    start=(ft == 0), stop=(ft == 15 and not tail))
            if tail:
                # z2 += o1 on the (tail-idle) PE; LN2 reads PSUM directly
                nc.tensor.matmul(p2[:], EYBcb, o1_sb[m][:],
                                 start=False, stop=True)
                z2 = p2
            else:
                z2t = wk.tile([128, D], BF16, tag="z2")
                nc.vector.tensor_add(z2t[:], p2[:], o1_sb[m][:])
                z2 = z2t
            mv, rsd = ln_rsd(z2)
            if trivial:
                yt = wk.tile([128, D], BF16, tag="yt")
                nc.vector.tensor_scalar(yt[:], z2[:], mv[:, 0:1], rsd[:],
                                        op0=ALU.subtract, op1=ALU.mult)
            else:
                nrm2 = wk.tile([128, D], BF16, tag="n2")
                nc.vector.tensor_scalar(nrm2[:], z2[:], mv[:, 0:1], rsd[:],
                                        op0=ALU.subtract, op1=ALU.mult)
                tg = wk.tile([128, D], BF16, tag="tg")
                yt = wk.tile([128, D], BF16, tag="yt")
                if tail:
                    nc.vector.tensor_mul(tg[:], nrm2[:], G2cb[:])
                    nc.vector.tensor_add(yt[:], tg[:], BE2cb[:])
                else:
                    nc.gpsimd.tensor_mul(tg[:], nrm2[:], G2cb[:])
                    nc.gpsimd.tensor_add(yt[:], tg[:], BE2cb[:])
            nc.sync.dma_start(out[m * 128:(m + 1) * 128, :], yt[:])

        # ---------------- schedule ----------------
        fill = lambda n: (lambda: warm_fill(n))
        uT_mm(0)
        warm_fill(6)
        uT_mm(1)
        warm_fill(10)
        ex0, rcp0 = attn_softmax(0)
        box = {}

        def mid_softmax1():
            # exp1 runs on the scalar queue right after the xas0 copy, so
            # it neither blocks xas0 nor waits behind the LN sqrts
            box["a1"] = attn_softmax(1)

        def mid_junk8():
            # preload the sqrt-family table set AFTER both Exps (reads ex1)
            junk8 = cst.tile([8, 1], F32, tag="junk8", name="junk8")
            nc.scalar.activation(junk8[:], box["a1"][0][:, 0:1], AF.Sqrt)

        las0 = attn_v(0, ex0, rcp0,
                      gaps=[fill(4), fill(4), fill(4), fill(4), fill(4),
                            fill(3)],
                      post_xas=mid_softmax1, post_vs=mid_junk8)
        warm_fill(3)
        for jt in range(4):
            attn_resid_bcp(0, las0, jt)
        for jt in range(4):
            attn_resid_fin(0, jt)
        warm_fill(8)
        attn_resid_tp(0, 0)
        warm_fill(3)
        attn_resid_tp(0, 1)
        warm_fill(3)
        attn_resid_tp(0, 2)
        attn_resid_tp(0, 3)

        def h1q(h, lo, hi, split):
            def go():
                for ft in range(lo, hi):
                    ffn_h1(h, ft, split=split)
            return go

        ex1, rcp1 = box["a1"]
        las1 = attn_v(1, ex1, rcp1,
                      gaps=[h1q(0, 0, 2, True), h1q(0, 2, 4, True),
                            h1q(0, 4, 6, True), h1q(0, 6, 8, True),
                            h1q(0, 8, 10, True), h1q(0, 10, 12, True)])
        for jt in range(4):
            attn_resid_bcp(1, las1, jt)
            if jt == 0:
                h1q(0, 12, 14, True)()
        for jt in range(4):
            attn_resid_fin(1, jt)
        h1q(0, 14, 16, True)()
        attn_resid_tp(1, 0)
        attn_resid_tp(1, 1)
        ffn_rm(0)
        attn_resid_tp(1, 2)
        attn_resid_tp(1, 3)
        h1q(1, 0, 4, False)()
        ffn_rm(1)
        h1q(1, 4, 8, False)()
        ffn_rm(2)
        h1q(1, 8, 12, False)()
        ffn_rm(3)
        h1q(1, 12, 16, False)()
        for m in range(4, 8):
            ffn_rm(m)

    nc.compile()
    return nc


def _pe_table():
    pos = np.arange(S, dtype=np.float32)[:, None]
    div = np.exp(np.arange(0, D, 2, dtype=np.float32) * (-math.log(10000.0) / D))
    ang = pos * div
    pe = np.zeros((S, D), np.float32)
    pe[:, 0::2] = np.sin(ang)
    pe[:, 1::2] = np.cos(ang)
    return pe


def _is_trivial(W1, b1, b2, g1, be1, g2, be2):
    f32 = lambda a: np.asarray(a, dtype=np.float32)
    return (np.all(f32(g1) == 1) and np.all(f32(be1) == 0)
            and np.all(f32(b2) == 0) and np.all(f32(g2) == 1)
            and np.all(f32(be2) == 0))


def make_in_maps(x, Wq, Wfc, W1, b1, W2, b2, g1, be1, g2, be2):
    f32 = lambda a: np.ascontiguousarray(a, dtype=np.float32)
    bfc = lambda a: np.ascontiguousarray(np.asarray(f32(a), dtype="bfloat16"))
    xf = f32(x).reshape(S * H * W, D)
    pe = _pe_table()
    M2 = f32(Wfc).reshape(D, NH, DEP).sum(axis=1).T          # (64, 512)
    Mstk = np.concatenate([M2, M2], axis=0)                  # (128, 512)

    WqTf = f32(Wq.T)
    Wu = WqTf.reshape(D, NH, DEP).sum(axis=2)                # (K, i)
    WUc = np.ascontiguousarray(
        Wu.reshape(4, 128, NH).transpose(1, 0, 2).reshape(128, 32),
        dtype=np.float16)

    CF = np.zeros((128, NCF), np.float32)
    CF[:, O_EYE:O_EYE + 128] = np.eye(128, dtype=np.float32)
    b1p = f32(b1) + f32(W1) @ f32(be1)                       # be1 folded
    CF[:, O_B1P:O_B1P + 16] = b1p.reshape(16, 128).T
    CF[:, O_WU:O_WU + 16] = WUc.view(np.float32)

    selTT = np.zeros((128, 8), np.float32)                   # [s, j]
    for p in range(128):
        selTT[p, p // 16] = 1.0

    CB = np.zeros((128, NCB), np.float32)
    CB[:, O_EYB:O_EYB + 128] = np.eye(128, dtype=np.float32)
    CB[:, O_MST:O_MST + D] = Mstk
    CB[:, O_SELB:O_SELB + 8] = selTT

    CS = np.zeros((8, NCS), np.float32)
    for jt in range(4):
        for p in range(128):
            CS[2 * jt + p // 64, O_E8 + jt * 128 + p] = 1.0
    CS[0, O_G1R:O_G1R + D] = f32(g1)
    CS[0, O_G2R:O_G2R + D] = f32(g2)
    CS[0, O_BE2R:O_BE2R + D] = f32(be2)
    CS[0, O_B1TR:O_B1TR + D] = f32(b2) + f32(be1)

    shared = dict(
        WqT=np.ascontiguousarray(WqTf, dtype=np.float16),
        W1T=bfc(f32(W1) * f32(g1)[None, :]).T.copy(),        # g1 folded
        W2T=bfc(f32(W2).T),
        CF=CF, CB=bfc(CB), CS8=bfc(CS),
    )
    selT = np.zeros((8, 128), np.float32)
    for rr in range(128):
        selT[rr // 16, rr] = 1.0

    peu_all = pe.reshape(S, NH, DEP).sum(axis=2)             # (S, 8) f32

    maps = []
    for k in range(NCORES):
        sl = xf[k * R:(k + 1) * R]
        m = dict(shared)
        slT = np.asarray(sl.T, dtype=np.float16)
        # xq layout: row-block i = m-pair (2i, 2i+1); columns (dt, mi, c)
        arr = slT.reshape(4, 128, 4, 2, 128)        # (t, p, i, mi, c)
        arr = arr.transpose(2, 1, 0, 3, 4)          # (i, p, t, mi, c)
        m["xT"] = np.ascontiguousarray(arr.reshape(512, 1024))
        # xRb[p, t*512+d] = x[t*128+p, d]
        m["xRb"] = bfc(sl.reshape(8, 128, D).transpose(1, 0, 2)
                       .reshape(128, 8 * D))
        cr8 = np.zeros((8, NCR8), np.float16)
        cr8[:, O_SEL:O_SEL + 128] = selT
        pe_loc = pe[k * 64:(k + 1) * 64]            # (64, 512)
        cr8[:, O_PET:O_PET + 8 * D] = (pe_loc.reshape(8, 8, D)
                                       .transpose(1, 0, 2).reshape(8, 8 * D))
        cr8[:, O_EY8:O_EY8 + 8] = np.eye(8, dtype=np.float16)
        # pe segment sums, exact f32, laid out [i, c*512 + st*128 + y]
        peu_loc = peu_all[k * 64:(k + 1) * 64]      # (64, 8)
        cu = np.zeros((8, 2 * D), np.float32)
        for mm in range(8):
            c, st = divmod(mm, 4)
            blk = peu_loc[mm * 8:(mm + 1) * 8]      # (j, i)
            cu[:, c * D + st * 128:c * D + (st + 1) * 128] = blk.T @ selT
        cr8[:, O_PEU:O_PEU + 4 * D] = cu.view(np.float16)
        m["CR8"] = np.ascontiguousarray(cr8)
        maps.append(m)
    return maps


def kernel(x, Wq, Wfc, W1, b1, W2, b2, g1, be1, g2, be2, _results_hook=None,
           _trace=False, _tmpdir=None):
    trivial = _is_trivial(W1, b1, b2, g1, be1, g2, be2)
    key = ("nc", trivial)
    if key not in _cached:
        _cached[key] = build_nc(trivial)
    nc = _cached[key]
    in_maps = make_in_maps(x, Wq, Wfc, W1, b1, W2, b2, g1, be1, g2, be2)
    res = run_bass_kernel_spmd(nc, in_maps, list(range(NCORES)),
                               trace=_trace, tmpdir=_tmpdir)
    if _results_hook is not None:
        _results_hook(res)
    y = np.concatenate([np.asarray(res.results[k]["out"], dtype=np.float32)
                        for k in range(NCORES)], axis=0)
    return y.reshape(S, H, W, D)
